# revision 30
# baseline (speedup 1.0000x reference)
"""Trainium2 Bass kernel for MultiHeadedSelfAttention with Shaw relative
position embeddings (clipped, R=64), sharded over 8 NeuronCores.

Sharding: core c handles batch b = c//4 and head group g = c%4 (4 heads).
Each core computes a partial output  ctx_g @ W_out[256g:256g+256]  for its
batch; the host sums the 4 partials per batch and adds b_out.

v3: transposed attention flow. Scores are computed directly in [k, q]
layout (lhsT = k-tile variants, rhs = q), so the AV matmul consumes exp
scores without transposing the full attention matrix (the old flow spent
1088 PE transposes + LDWEIGHTS on that). Only the 384-wide W-band around
the diagonal is additionally computed in [q, k] layout, where the
softmax statistics need it: masked partial tail sums, and the diagonal
reads for the rel-v interior coefficients. Full-row statistics
(denominator D, per-k-tile sums T_kt, prefix subsets) come free as extra
output rows of the AV matmul via an augmented V operand
[v | ones | eye16 | subL | subR]  (M=113).
"""
import sys

sys.path.insert(0, "/opt/trn_rl_repo")

import numpy as np

B, S, D, H, RR, VOC = 2, 2048, 1024, 16, 64, 129
HD = 64              # head dim
NH = 4               # heads per core
N_CORES = 8
NT = S // 128        # 16 q-tiles of 128
IMW = 512            # qrel image width (clip-padded)
IMWW = 448           # exp W-zone image width (64-col front pad)
FPAD = 64            # front pad of the exp image
SCALE = 0.125        # 1/sqrt(64)
MAV = 113            # AV lhsT cols: 64 v + 1 ones + 16 eye + 16 subL + 16 subR

_cache = {}


def _regions(t):
    """W-band bounds (in k) for q-tile t."""
    i0 = 128 * t
    wlo = max(0, i0 - 128)
    whi = 256 if t == 0 else min(S, i0 + 256)
    return i0, wlo, whi


def _build():
    import concourse.bass as bass
    import concourse.mybir as mybir
    import concourse.tile as tile
    from concourse import bacc
    from contextlib import ExitStack

    F32 = mybir.dt.float32
    F32R = mybir.dt.float32r
    F16 = mybir.dt.float16
    AP = bass.AP
    AF = mybir.ActivationFunctionType
    ALU = mybir.AluOpType
    AX = mybir.AxisListType

    nc = bacc.Bacc("TRN2", target_bir_lowering=False, debug=False,
                   num_devices=N_CORES)

    # ---------------- DRAM I/O ----------------
    xT = nc.dram_tensor("xT", [D, S], F32, kind="ExternalInput").ap()
    wq = nc.dram_tensor("wq", [D, 256], F32, kind="ExternalInput").ap()
    wk = nc.dram_tensor("wk", [D, 256], F32, kind="ExternalInput").ap()
    wv = nc.dram_tensor("wv", [D, 256], F32, kind="ExternalInput").ap()
    bq = nc.dram_tensor("bq", [128, 2], F32, kind="ExternalInput").ap()
    bk = nc.dram_tensor("bk", [128, 2], F32, kind="ExternalInput").ap()
    r01 = nc.dram_tensor("r01", [128, 2], F32, kind="ExternalInput").ap()
    relk = nc.dram_tensor("relk", [128, 512], F16, kind="ExternalInput").ap()
    rvm2 = nc.dram_tensor("rvm2", [128, 64], F16, kind="ExternalInput").ap()
    rv0r = nc.dram_tensor("rv0r", [1, 64], F16, kind="ExternalInput").ap()
    rvl = nc.dram_tensor("rvl", [1, 64], F16, kind="ExternalInput").ap()
    wout = nc.dram_tensor("wout", [128, 2, 1024], F16, kind="ExternalInput").ap()
    mlm = nc.dram_tensor("mlm", [128, 512], F16, kind="ExternalInput").ap()
    mrm = nc.dram_tensor("mrm", [128, 512], F16, kind="ExternalInput").ap()
    ident = nc.dram_tensor("ident", [128, 128], F16, kind="ExternalInput").ap()
    vaug = nc.dram_tensor("vaug", [128, 16 * NH * 49], F16,
                          kind="ExternalInput").ap()
    steps = nc.dram_tensor("steps", [128, 16], F32, kind="ExternalInput").ap()
    bvbc = nc.dram_tensor("bvbc", [64, 1024], F16, kind="ExternalInput").ap()
    out = nc.dram_tensor("out", [S, D], F16, kind="ExternalOutput").ap()
    import os
    DBG = os.environ.get("KDBG", "") == "1"
    if DBG:
        dbg_av = nc.dram_tensor("dbg_av", [128, 1024], F32,
                                kind="ExternalOutput").ap()
        dbg_row = nc.dram_tensor("dbg_row", [1, 128 * 24], F16,
                                 kind="ExternalOutput").ap()
        dbg_stg = nc.dram_tensor("dbg_stg", [128, NH * 8 * 128], F16,
                                 kind="ExternalOutput").ap()
        dbg_ar = nc.dram_tensor("dbg_ar", [128, 2 * 512], F16,
                                kind="ExternalOutput").ap()
        dbg_et = nc.dram_tensor("dbg_et", [128, 4 * 512], F16,
                                kind="ExternalOutput").ap()
        DBG_G = 3

    # DRAM scratch images
    imgq_t = nc.dram_tensor("imgq", [NT * NH * 128 * IMW], F16)   # qrel pad
    imgw_t = nc.dram_tensor("imgw", [NT * NH * 128 * IMWW], F16)  # exp W-band
    statd_t = nc.dram_tensor("statd", [2 * 49 * 1024], F32)       # stat rows

    def qbase(t):
        return t * NH * 128 * IMW

    def wbase(t, h=0):
        return (t * NH + h) * 128 * IMWW

    with tile.TileContext(nc) as tc, ExitStack() as ctx:
        # ---------------- persistent pools ----------------
        pp = ctx.enter_context(tc.tile_pool(name="persist", bufs=1))
        qkT = []   # per pair: qT16, kW16, kL16, kR16  [128, S] fp16
        for pair in range(2):
            qkT.append({
                "q": pp.tile([128, S], F16, tag=f"qT{pair}", name=f"qT{pair}"),
                "W": pp.tile([128, S], F16, tag=f"kW{pair}", name=f"kW{pair}"),
                "L": pp.tile([128, S], F16, tag=f"kL{pair}", name=f"kL{pair}"),
                "R": pp.tile([128, S], F16, tag=f"kR{pair}", name=f"kR{pair}"),
            })
        # augmented V: [j, kt, h, 113] = [v_h | ones | eye16 | subL | subR]
        vA = pp.tile([128, NT, NH, MAV], F16, tag="vA", name="vA")
        relk_sb = pp.tile([128, 512], F16, tag="relk", name="relk")
        rvm2_sb = pp.tile([128, 64], F16, tag="rvm2", name="rvm2")
        rv0r_sb = pp.tile([1, 64], F16, tag="rv0r", name="rv0r")
        rvl_sb = pp.tile([1, 64], F16, tag="rvl", name="rvl")
        wout_sb = pp.tile([128, 2, 1024], F16, tag="wout", name="wout")
        bq_sb = pp.tile([128, 2], F32, tag="bq", name="bq")
        bk_sb = pp.tile([128, 2], F32, tag="bk", name="bk")
        r01_sb = pp.tile([128, 2], F32, tag="r01", name="r01")
        ml_sb = pp.tile([128, 512], F16, tag="ml", name="ml")
        mr_sb = pp.tile([128, 512], F16, tag="mr", name="mr")
        id_sb = pp.tile([128, 128], F16, tag="ident", name="ident")
        steps_sb = pp.tile([128, 2, 8], F32, tag="steps", name="steps")
        bvbc_sb = pp.tile([64, 2, 2, 256], F16, tag="bvbc", name="bvbc")
        ones1 = pp.tile([1, 128], F16, tag="ones1", name="ones1")
        zeros16 = pp.tile([128, 128], F16, tag="zeros16", name="zeros16")

        # urgent loads (phase 1 deps) on the sync queue; everything else
        # on the scalar HWDGE queue so it doesn't delay the x/W loads
        nc.sync.dma_start(bq_sb[:], bq)
        nc.sync.dma_start(bk_sb[:], bk)
        nc.sync.dma_start(r01_sb[:], r01)
        nc.scalar.dma_start(relk_sb[:], relk)
        nc.scalar.dma_start(rvm2_sb[:], rvm2)
        nc.scalar.dma_start(rv0r_sb[:], rv0r)
        nc.scalar.dma_start(rvl_sb[:], rvl)
        nc.scalar.dma_start(wout_sb[:], wout)
        nc.scalar.dma_start(ml_sb[:], mlm)
        nc.scalar.dma_start(mr_sb[:], mrm)
        nc.scalar.dma_start(id_sb[:], ident)
        nc.scalar.dma_start(vA[:, :, :, 64:MAV], vaug)
        nc.scalar.dma_start(steps_sb[:], steps)
        nc.scalar.dma_start(bvbc_sb[:], bvbc)
        nc.gpsimd.memset(ones1[:], 1.0)
        nc.gpsimd.memset(zeros16[:], 0.0)
        # zero the t=0 front pad (cols [0,64)) and t=15 back pad
        # (cols [320,448)) of the exp images so diagonal reads see 0
        for h in range(NH):
            nc.scalar.dma_start(
                AP(imgw_t, wbase(0, h), [[IMWW, 128], [1, FPAD]]),
                zeros16[:, 0:FPAD])
            nc.scalar.dma_start(
                AP(imgw_t, wbase(15, h) + FPAD + 256,
                   [[IMWW, 128], [1, 128]]),
                zeros16[:, 0:128])

        # ---------------- phase 1: projections ----------------
        with tc.tile_pool(name="p1", bufs=1) as p1, \
             tc.tile_pool(name="p1ps", bufs=2, space="PSUM") as p1ps:
            xT_sb = p1.tile([128, 8, S], F32R, tag="xT", name="xT")
            wq_sb = p1.tile([128, 8, 256], F32R, tag="wq", name="wq")
            wk_sb = p1.tile([128, 8, 256], F32R, tag="wk", name="wk")
            wv_sb = p1.tile([128, 8, 256], F32R, tag="wv", name="wv")
            xTr = xT.rearrange("(c p) s -> p c s", p=128).bitcast(F32R)
            nc.sync.dma_start(wq_sb[:], wq.rearrange("(c p) n -> p c n", p=128).bitcast(F32R))
            nc.sync.dma_start(xT_sb[:, :, 0:512], xTr[:, :, 0:512])
            nc.sync.dma_start(wk_sb[:], wk.rearrange("(c p) n -> p c n", p=128).bitcast(F32R))
            for sc in range(1, 4):
                nc.sync.dma_start(xT_sb[:, :, 512 * sc:512 * sc + 512],
                                  xTr[:, :, 512 * sc:512 * sc + 512])
            nc.sync.dma_start(wv_sb[:], wv.rearrange("(c p) n -> p c n", p=128).bitcast(F32R))

            # q, k (transposed layout [col, s]) per pair
            for pair in range(2):
                for sc in range(4):  # s-chunks of 512
                    ps_q = p1ps.tile([128, 512], F32, tag="p1q", name="p1q")
                    ps_k = p1ps.tile([128, 512], F32, tag="p1k", name="p1k")
                    for dk in range(8):
                        nc.tensor.matmul(
                            ps_q[:], wq_sb[:, dk, 128 * pair:128 * pair + 128],
                            xT_sb[:, dk, 512 * sc:512 * sc + 512],
                            start=(dk == 0), stop=(dk == 7))
                        nc.tensor.matmul(
                            ps_k[:], wk_sb[:, dk, 128 * pair:128 * pair + 128],
                            xT_sb[:, dk, 512 * sc:512 * sc + 512],
                            start=(dk == 0), stop=(dk == 7))
                    cs = slice(512 * sc, 512 * sc + 512)
                    nc.vector.tensor_scalar(
                        qkT[pair]["q"][:, cs], ps_q[:], bq_sb[:, pair:pair + 1],
                        SCALE, op0=ALU.add, op1=ALU.mult)
                    nc.vector.tensor_scalar_add(
                        qkT[pair]["W"][:, cs], ps_k[:], bk_sb[:, pair:pair + 1])
                    nc.vector.tensor_scalar_add(
                        qkT[pair]["L"][:, cs], qkT[pair]["W"][:, cs],
                        r01_sb[:, 0:1])
                    nc.vector.tensor_scalar_add(
                        qkT[pair]["R"][:, cs], qkT[pair]["W"][:, cs],
                        r01_sb[:, 1:2])

            # v (natural layout [s, col]) into vA content columns
            for st in range(NT):
                ps_v = p1ps.tile([128, 256], F32, tag="p1v", name="p1v")
                for dk in range(8):
                    nc.tensor.matmul(
                        ps_v[:], xT_sb[:, dk, 128 * st:128 * st + 128],
                        wv_sb[:, dk, :], start=(dk == 0), stop=(dk == 7))
                nc.vector.tensor_copy(
                    vA[:, st, :, 0:64],
                    ps_v[:].rearrange("p (h d) -> p h d", h=NH))

        # ---------------- phase 1.5: qrel images for all tiles ----------
        with tc.tile_pool(name="qrp", bufs=2) as qrp, \
             tc.tile_pool(name="qrps", bufs=1, space="PSUM") as qrps:
            for t in range(NT):
                qrelpad = qrp.tile([128, NH, IMW], F16, tag="qrelpad",
                                   name="qrelpad")
                for pair in range(2):
                    for h01 in range(2):
                        h = 2 * pair + h01
                        rs = slice(64 * h01, 64 * h01 + 64)
                        qr = qrps.tile([128, 512], F32, tag="qr", name="qr")
                        nc.tensor.matmul(
                            qr[:], qkT[pair]["q"][rs, 128 * t:128 * t + 128],
                            relk_sb[rs, :], start=True, stop=True)
                        if h % 2 == 0:
                            nc.vector.tensor_copy(qrelpad[:, h, :], qr[:])
                        else:
                            nc.scalar.activation(qrelpad[:, h, :], qr[:],
                                                 AF.Copy)
                nc.sync.dma_start(
                    AP(imgq_t, qbase(t),
                       [[IMW, 128], [128 * IMW, NH], [1, IMW]]),
                    qrelpad[:])

        # ---------------- phase 2 pools ----------------
        stgp = ctx.enter_context(tc.tile_pool(name="stgp", bufs=2))
        bndp = ctx.enter_context(tc.tile_pool(name="bndp", bufs=2))
        expwp = ctx.enter_context(tc.tile_pool(name="expwp", bufs=9))
        scrp = ctx.enter_context(tc.tile_pool(name="scrp", bufs=4))
        arp = ctx.enter_context(tc.tile_pool(name="arp", bufs=6))
        etp = ctx.enter_context(tc.tile_pool(name="etp", bufs=3))
        stp = ctx.enter_context(tc.tile_pool(name="stp", bufs=2))
        colp = ctx.enter_context(tc.tile_pool(name="colp", bufs=2))
        atsp = ctx.enter_context(tc.tile_pool(name="atsp", bufs=2))
        ctp = ctx.enter_context(tc.tile_pool(name="ctp", bufs=2))
        ct16p = ctx.enter_context(tc.tile_pool(name="ct16p", bufs=2))
        outp = ctx.enter_context(tc.tile_pool(name="outp", bufs=2))
        # PSUM: av 2 banks x2, qe 1 bank x2, wqps 1 bank x2 = 8 banks
        avp = ctx.enter_context(tc.tile_pool(name="avp", bufs=2, space="PSUM"))
        qep = ctx.enter_context(tc.tile_pool(name="qep", bufs=2, space="PSUM"))
        wqp = ctx.enter_context(tc.tile_pool(name="wqp", bufs=2, space="PSUM"))

        def emit_wstage(g):
            """q-layout W-band for q-tiles 2g, 2g+1: exp, tails, imgw write,
            PE transposes into staging for the transposed AV flow."""
            stg = stgp.tile([128, NH, 4, 2, 128], F16, tag="stg", name="stg")
            pkL = colp.tile([128, 8], F32, tag="pkL", name="pkL")
            pkR = colp.tile([128, 8], F32, tag="pkR", name="pkR")
            arel16s = []
            expws = {}
            # pass 1: scores + band + exp + tails (psum freed at the DVE add)
            for tq in range(2):
                t = 2 * g + tq
                i0, wlo, whi = _regions(t)
                wlen = whi - wlo
                moff = 128 if t == 0 else 0
                band4 = bndp.tile([128, NH, 384], F16, tag="band4",
                                  name="band4")
                nc.sync.dma_start(
                    band4[:, :, 0:wlen],
                    AP(imgq_t, qbase(t) + 256 - (i0 - wlo),
                       [[IMW - 1, 128], [128 * IMW, NH], [1, wlen]]))
                for pair in range(2):
                    for h01 in range(2):
                        h = 2 * pair + h01
                        u = 4 * tq + 2 * pair + h01
                        rs = slice(64 * h01, 64 * h01 + 64)
                        wqps = wqp.tile([128, 384], F32, tag="wqps",
                                        name="wqps")
                        nc.tensor.matmul(
                            wqps[:, 0:wlen],
                            qkT[pair]["q"][rs, 128 * t:128 * t + 128],
                            qkT[pair]["W"][rs, wlo:whi],
                            start=True, stop=True)
                        scW = scrp.tile([128, 384], F16, tag="scW",
                                        name="scW")
                        nc.vector.tensor_add(
                            scW[:, 0:wlen], wqps[:, 0:wlen],
                            band4[:, h, 0:wlen])
                        expw = expwp.tile([128, 384], F16, tag="expw",
                                          name="expw")
                        nc.scalar.activation(expw[:, 0:wlen], scW[:, 0:wlen],
                                             AF.Exp)
                        expws[(tq, h)] = expw
                        nc.sync.dma_start(
                            AP(imgw_t, wbase(t, h) + FPAD,
                               [[IMWW, 128], [1, wlen]]),
                            expw[:, 0:wlen])
                        # masked partial tails -> pk[:, u]
                        scr = scrp.tile([128, 384], F16, tag="scr", name="scr")
                        nc.gpsimd.tensor_tensor(
                            scr[:, 0:wlen], expw[:, 0:wlen],
                            ml_sb[:, moff:moff + wlen], op=ALU.mult)
                        nc.vector.tensor_reduce(
                            pkL[:, u:u + 1], scr[:, 0:wlen], axis=AX.X,
                            op=ALU.add)
                        scr2 = scrp.tile([128, 384], F16, tag="scr",
                                         name="scr2")
                        nc.gpsimd.tensor_tensor(
                            scr2[:, 0:wlen], expw[:, 0:wlen],
                            mr_sb[:, moff:moff + wlen], op=ALU.mult)
                        nc.vector.tensor_reduce(
                            pkR[:, u:u + 1], scr2[:, 0:wlen], axis=AX.X,
                            op=ALU.add)
                arel16 = arp.tile([128, NH, 127], F16, tag="arel16",
                                  name="arel16")
                nc.sync.dma_start(
                    arel16[:],
                    AP(imgw_t, wbase(t) + (i0 - wlo) + 1,
                       [[IMWW + 1, 128], [128 * IMWW, NH], [1, 127]]))
                arel16s.append(arel16)
            # pass 2: PE transposes into staging (exps are long done)
            for tq in range(2):
                t = 2 * g + tq
                i0, wlo, whi = _regions(t)
                kt0 = wlo // 128
                ktrel0 = kt0 - (2 * g - 1)
                nb = (whi - wlo) // 128
                for h in range(NH):
                    expw = expws[(tq, h)]
                    tp = wqp.tile([128, 384], F16, tag="wqps", name="tp")
                    for b in range(nb):
                        nc.tensor.transpose(
                            tp[:, 128 * b:128 * b + 128],
                            expw[:, 128 * b:128 * b + 128], id_sb[:])
                    nc.vector.tensor_copy(
                        stg[:, h, ktrel0:ktrel0 + nb, tq, :],
                        tp[:, 0:128 * nb].rearrange("p (b c) -> p b c", b=nb))
            return {"stg": stg, "pkL": pkL, "pkR": pkR, "arel": arel16s}

        def emit_ktloop(g, wctx):
            """Transposed scores (far zones) + exp + AV accumulation."""
            g0 = 256 * g
            stg = wctx["stg"]
            av = avp.tile([128, 2, 2, 256], F32, tag="av", name="av")
            pend_av = []   # AV jobs delayed one chunk behind scores/exp

            def flush_av():
                for kt, pair, h01, rhs in pend_av:
                    h = 2 * pair + h01
                    nc.tensor.matmul(
                        av[0:MAV, pair, h01, :], vA[:, kt, h, :], rhs,
                        start=(kt == 0 and h01 == 0),
                        stop=(kt == 15), skip_group_check=True)
                pend_av.clear()

            for m in range(8):
                for pair in range(2):
                    for h01 in range(2):
                        h = 2 * pair + h01
                        if m == g:
                            flush_av()
                            for s2 in range(2):
                                pend_av.append(
                                    (2 * m + s2, pair, h01,
                                     stg[:, h, 1 + s2, :, :]))
                            continue
                        rs = slice(64 * h01, 64 * h01 + 64)
                        qx = qep.tile([128, 2, 256], F32, tag="qe", name="qx")
                        et = etp.tile([128, 2, 256], F16, tag="expT",
                                      name="et")
                        fss = []
                        for s2 in range(2):
                            kt = 2 * m + s2
                            var = "L" if kt < 2 * g else "R"
                            if kt == 2 * g - 1:
                                fs = slice(128, 256)   # far cols of group
                            elif kt == 2 * g + 2:
                                fs = slice(0, 128)
                            else:
                                fs = slice(0, 256)
                            fss.append(fs)
                            nc.tensor.matmul(
                                qx[:, s2, fs],
                                qkT[pair][var][rs, 128 * kt:128 * kt + 128],
                                qkT[pair]["q"][rs, g0 + fs.start:g0 + fs.stop],
                                start=True, stop=True)
                        flush_av()
                        if fss[0] == slice(0, 256) and fss[1] == slice(0, 256):
                            nc.scalar.activation(et[:], qx[:], AF.Exp)
                        else:
                            for s2 in range(2):
                                nc.scalar.activation(
                                    et[:, s2, fss[s2]], qx[:, s2, fss[s2]],
                                    AF.Exp)
                        for s2 in range(2):
                            kt = 2 * m + s2
                            if kt == 2 * g - 1:
                                nc.vector.tensor_copy(
                                    et[:, s2, 0:128], stg[:, h, 0, 0, :])
                            elif kt == 2 * g + 2:
                                nc.vector.tensor_copy(
                                    et[:, s2, 128:256], stg[:, h, 3, 1, :])
                        if DBG and g == DBG_G and m == 0:
                            nc.sync.dma_start(
                                dbg_et[:, 512 * h:512 * h + 512],
                                et.rearrange("p a b -> p (a b)"))
                        for s2 in range(2):
                            pend_av.append((2 * m + s2, pair, h01,
                                            et[:, s2, :]))
            flush_av()
            return av

        def emit_finish(g, wctx, av):
            """Stats algebra, rel-v, normalization, output projection."""
            pkL, pkR = wctx["pkL"], wctx["pkR"]
            arel16s = wctx["arel"]
            # stats rows -> DRAM, then strided reads flip them to columns
            stats = stp.tile([128, 2, 2, 256], F32, tag="stats", name="stats")
            nc.vector.tensor_copy(stats[64:MAV], av[64:MAV])
            sb0 = (g & 1) * 49 * 1024
            nc.sync.dma_start(
                AP(statd_t, sb0, [[1024, 49], [1, 1024]]), stats[64:MAV])
            Dcol = colp.tile([128, 8], F32, tag="Dcol", name="Dcol")
            TcolL = colp.tile([128, 8], F32, tag="TcolL", name="TcolL")
            TcolR = colp.tile([128, 8], F32, tag="TcolR", name="TcolR")
            subL8 = colp.tile([128, 8], F32, tag="subL8", name="subL8")
            subR8 = colp.tile([128, 8], F32, tag="subR8", name="subR8")
            for tq in range(2):
                t = 2 * g + tq
                # statd row r: 0=D, 1+kt=T_kt, 17+t=subL_t, 33+t=subR_t
                rL = t if t > 0 else 0        # T_{t-1}; junk for t=0
                rRr = t + 2 if t < 15 else 0  # T_{t+1}; junk for t=15
                for dst, r in ((Dcol, 0), (TcolL, rL), (TcolR, rRr),
                               (subL8, 17 + t), (subR8, 33 + t)):
                    nc.sync.dma_start(
                        dst[:, 4 * tq:4 * tq + 4].rearrange(
                            "p (a b) -> p a b", a=2),
                        AP(statd_t, sb0 + r * 1024 + 128 * tq,
                           [[1, 128], [512, 2], [256, 2]]))
            sL8 = colp.tile([128, 8], F32, tag="sL8", name="sL8")
            sR8 = colp.tile([128, 8], F32, tag="sR8", name="sR8")
            nc.vector.tensor_tensor(sL8[:], TcolL[:], steps_sb[:, 0, :],
                                    op=ALU.mult)
            nc.vector.tensor_add(sL8[:], sL8[:], subL8[:])
            nc.vector.tensor_add(sL8[:], sL8[:], pkL[:])
            nc.vector.tensor_tensor(sR8[:], TcolR[:], steps_sb[:, 1, :],
                                    op=ALU.mult)
            nc.vector.tensor_add(sR8[:], sR8[:], subR8[:])
            nc.vector.tensor_add(sR8[:], sR8[:], pkR[:])
            if g == 0:   # t=0 has no T_{t-1} term
                nc.vector.tensor_tensor(sL8[:, 0:4], subL8[:, 0:4],
                                        pkL[:, 0:4], op=ALU.add)
            if g == 7:   # t=15 has no T_{t+1} term
                nc.vector.tensor_tensor(sR8[:, 4:8], subR8[:, 4:8],
                                        pkR[:, 4:8], op=ALU.add)
            rec = colp.tile([128, 8], F32, tag="rec", name="rec")
            nc.vector.reciprocal(rec[:], Dcol[:])
            # pack to fp16 rows: chans [rec | sL | sR], each (pr, h01, tq)
            pkout = colp.tile([128, 24], F16, tag="pkout", name="pkout")
            for blk, src in ((0, rec), (8, sL8), (16, sR8)):
                nc.vector.tensor_copy(
                    pkout[:, blk:blk + 8].rearrange(
                        "p (a b c) -> p c a b", a=2, b=2),
                    src[:].rearrange("p (c a b) -> p c a b", c=2, a=2))
            rowout = colp.tile([1, 128, 24], F16, tag="rowout", name="rowout")
            nc.sync.dma_start(rowout.rearrange("o p c -> o (p c)"), pkout[:])
            if DBG and g == DBG_G:
                nc.vector.tensor_copy(stats[0:64], av[0:64])
                nc.sync.dma_start(dbg_av,
                                  stats.rearrange("p a b c -> p (a b c)"))
                nc.sync.dma_start(dbg_row, rowout.rearrange("o p c -> o (p c)"))
                nc.sync.dma_start(
                    dbg_stg,
                    wctx["stg"].rearrange("p a b c d -> p (a b c d)"))

            ct16 = {}
            for pair in range(2):
                # rel-v interior via transposed diagonal coefficients
                atp = wqp.tile([127, 2, 2, 128], F16, tag="wqps", name="atp")
                for h01 in range(2):
                    for tq in range(2):
                        nc.tensor.transpose(
                            atp[0:127, h01, tq, :],
                            arel16s[tq][:, 2 * pair + h01, 0:127], id_sb[:])
                arelTs = atsp.tile([127, 2, 2, 128], F16, tag="arelTs",
                                   name="arelTs")
                nc.vector.tensor_copy(arelTs[0:127], atp[0:127])
                if DBG and g == DBG_G:
                    nc.sync.dma_start(
                        dbg_ar[0:127, 512 * pair:512 * pair + 512],
                        arelTs.rearrange("p a b c -> p (a b c)"))
                # rel-v terms accumulate straight into av rows 0..63
                nc.tensor.matmul(
                    av[0:64, pair, :, :], rvm2_sb[0:127, :],
                    arelTs[0:127].rearrange("p a b c -> p (a b c)"),
                    start=False, stop=False, skip_group_check=True)
                nc.tensor.matmul(
                    av[0:64, pair, :, :], rv0r_sb[:],
                    rowout[0:1, :, 8 + 4 * pair:12 + 4 * pair].rearrange(
                        "o p (a b) -> o a b p", a=2),
                    start=False, stop=False, skip_group_check=True)
                nc.tensor.matmul(
                    av[0:64, pair, :, :], rvl_sb[:],
                    rowout[0:1, :, 16 + 4 * pair:20 + 4 * pair].rearrange(
                        "o p (a b) -> o a b p", a=2),
                    start=False, stop=True, skip_group_check=True)
                bcps = qep.tile([64, 2, 256], F32, tag="qe", name="bcps")
                nc.tensor.matmul(
                    bcps[:], ones1[0:1, 0:64],
                    rowout[0:1, :, 4 * pair:4 * pair + 4].rearrange(
                        "o p (a b) -> o a b p", a=2),
                    start=True, stop=True)
                rbc = ctp.tile([64, 2, 256], F16, tag="rbc", name="rbc")
                nc.vector.tensor_copy(rbc[:], bcps[:])
                ctmp = ctp.tile([64, 2, 256], F16, tag="ctmp", name="ctmp")
                nc.vector.tensor_tensor(ctmp[:], av[0:64, pair, :, :], rbc[:],
                                        op=ALU.mult)
                nc.vector.tensor_add(ctmp[:], ctmp[:], bvbc_sb[:, pair, :, :])
                ct = ct16p.tile([128, 256], F16, tag=f"ct{pair}",
                                name=f"ct{pair}")
                nc.vector.tensor_copy(ct[0:64, :], ctmp[:, 0, :])
                nc.sync.dma_start(ct[64:128, :], ctmp[:, 1, :])
                ct16[pair] = ct

            for tq in range(2):
                out_sb = outp.tile([128, 1024], F16, tag="out_sb",
                                   name="out_sb")
                for nch in range(2):
                    op_ps = qep.tile([128, 512], F32, tag="qe", name="op_ps")
                    for pair in range(2):
                        nc.tensor.matmul(
                            op_ps[:], ct16[pair][:, 128 * tq:128 * tq + 128],
                            wout_sb[:, pair, 512 * nch:512 * nch + 512],
                            start=(pair == 0), stop=(pair == 1))
                    nc.vector.tensor_copy(
                        out_sb[:, 512 * nch:512 * nch + 512], op_ps[:])
                r0_ = 256 * g + 128 * tq
                nc.sync.dma_start(out[r0_:r0_ + 128, :], out_sb[:])

        # ---------------- phase 2: software-pipelined groups -------------
        # finish(g) trails ktloop(g+1) so its serial stats chain hides
        # under the next group's tensor work
        wctx = emit_wstage(0)
        pend = None   # (g, wctx, av) awaiting finish
        for g in range(8):
            av = emit_ktloop(g, wctx)
            cur = (g, wctx, av)
            if g < 7:
                wctx = emit_wstage(g + 1)
            if pend is not None:
                emit_finish(*pend)
            pend = cur
        emit_finish(*pend)

    nc.compile()
    return nc


def get_nc():
    if "nc" not in _cache:
        _cache["nc"] = _build()
    return _cache["nc"]


def shard_inputs(inputs):
    """Build per-core input maps from full inputs (layout prep only)."""
    x = np.asarray(inputs["x"], np.float32)
    W_qkv = np.asarray(inputs["W_qkv"], np.float32)
    b_qkv = np.asarray(inputs["b_qkv"], np.float32)
    W_out = np.asarray(inputs["W_out"], np.float32)
    rk = np.asarray(inputs["rel_emb_k"], np.float32)
    rv = np.asarray(inputs["rel_emb_v"], np.float32)

    Wq, Wk, Wv = W_qkv[:, 0:D], W_qkv[:, D:2 * D], W_qkv[:, 2 * D:3 * D]
    bqf, bkf, bvf = b_qkv[0:D], b_qkv[D:2 * D], b_qkv[2 * D:3 * D]

    cidx = np.clip(np.arange(512) - 256, -64, 64) + 64   # [512] vocab index
    relk_pad = rk.T[:, cidx].astype(np.float16)           # [64, 512]
    relk_host = np.concatenate([relk_pad, relk_pad], axis=0)  # [128, 512]
    rvm2_host = np.zeros((128, 64), np.float16)
    rvm2_host[0:127] = rv[1:128].astype(np.float16)
    rv0r_host = rv[0:1].astype(np.float16)
    rvl_host = rv[128:129].astype(np.float16)
    r0 = np.tile(rk[0], 2).reshape(128, 1)
    r1 = np.tile(rk[128], 2).reshape(128, 1)
    r01_host = np.concatenate([r0, r1], 1).astype(np.float32)

    # tail masks [128, 512] master: middle tiles slice [0:wlen],
    # t=0 slices [128:128+wlen]
    jj = np.arange(512)[None, :]
    ppi = np.arange(128)[:, None]
    ml_host = ((jj <= ppi + 64) &
               (jj >= 128 * (ppi >= 64))).astype(np.float16)
    mr_host = ((jj >= ppi + 192) &
               (jj <= 255 + 128 * (ppi >= 64))).astype(np.float16)
    ident_host = np.eye(128, dtype=np.float16)

    # augmented-V constant columns [kt, h, 49]:
    # col 0: ones (D); 1..16: eye16 (T_kt); 17..32: subL; 33..48: subR
    aug = np.zeros((16, 49), np.float16)
    aug[:, 0] = 1.0
    for kt in range(16):
        aug[kt, 1 + kt] = 1.0
        for t in range(16):
            if kt <= t - 2:
                aug[kt, 17 + t] = 1.0
            if kt >= t + 2:
                aug[kt, 33 + t] = 1.0
    vaug_host = np.broadcast_to(
        np.repeat(aug[:, None, :], NH, axis=1).reshape(1, -1),
        (128, 16 * NH * 49)).astype(np.float16).copy()

    # step gates for the boundary T_kt term: L uses i_loc>=64, R uses <64
    iloc = np.arange(128)[:, None]
    st_hi = (iloc >= 64).astype(np.float32)
    st_lo = (iloc < 64).astype(np.float32)
    steps_host = np.concatenate(
        [np.repeat(st_hi, 8, 1), np.repeat(st_lo, 8, 1)], 1)

    in_maps = []
    for c in range(N_CORES):
        b, gg = c // 4, c % 4
        cols = slice(256 * gg, 256 * gg + 256)
        bvc = bvf[cols].reshape(2, 2, 64)          # [pair, h01, hd]
        bvbc_host = np.broadcast_to(
            bvc.transpose(2, 0, 1)[:, :, :, None],
            (64, 2, 2, 256)).reshape(64, 1024).astype(np.float16).copy()
        m = {
            "xT": np.ascontiguousarray(x[b].T),
            "wq": np.ascontiguousarray(Wq[:, cols]),
            "wk": np.ascontiguousarray(Wk[:, cols]),
            "wv": np.ascontiguousarray(Wv[:, cols]),
            "bq": np.ascontiguousarray(bqf[cols].reshape(2, 128).T),
            "bk": np.ascontiguousarray(bkf[cols].reshape(2, 128).T),
            "r01": r01_host,
            "relk": relk_host,
            "rvm2": rvm2_host,
            "rv0r": rv0r_host,
            "rvl": rvl_host,
            "wout": np.ascontiguousarray(
                W_out[cols].reshape(2, 128, 1024).transpose(1, 0, 2)
            ).astype(np.float16),
            "mlm": ml_host,
            "mrm": mr_host,
            "ident": ident_host,
            "vaug": vaug_host,
            "steps": steps_host,
            "bvbc": bvbc_host,
        }
        in_maps.append(m)
    return in_maps


def unshard_outputs(results, inputs):
    b_out = np.asarray(inputs["b_out"], np.float32)
    out = np.zeros((B, S, D), np.float32)
    for c in range(N_CORES):
        out[c // 4] += results[c]["out"].astype(np.float32)
    out += b_out[None, None, :]
    return out


def kernel(**inputs):
    from concourse import bass_utils
    nc = get_nc()
    in_maps = shard_inputs(inputs)
    res = bass_utils.run_bass_kernel_spmd(nc, in_maps, list(range(N_CORES)))
    return unshard_outputs(res.results, inputs)


if __name__ == "__main__":
    rng = np.random.default_rng(0)
    demo = {
        "x": rng.standard_normal((B, S, D)).astype(np.float32),
        "W_qkv": (rng.standard_normal((D, 3 * D)) * 0.02).astype(np.float32),
        "b_qkv": np.zeros(3 * D, np.float32),
        "W_out": (rng.standard_normal((D, D)) * 0.02).astype(np.float32),
        "b_out": np.zeros(D, np.float32),
        "rel_emb_k": (rng.standard_normal((VOC, HD)) * 0.02).astype(np.float32),
        "rel_emb_v": (rng.standard_normal((VOC, HD)) * 0.02).astype(np.float32),
    }
    o = kernel(**demo)
    print(o.shape, float(np.abs(o).max()))


# revision 40
# speedup vs baseline: 1.1462x; 1.1462x over previous
"""Trainium2 Bass kernel for MultiHeadedSelfAttention with Shaw relative
position embeddings (clipped, R=64), sharded over 8 NeuronCores.

Sharding: core c handles batch b = c//4 and head group g = c%4 (4 heads).
Each core computes a partial output  ctx_g @ W_out[256g:256g+256]  for its
batch; the host sums the 4 partials per batch and adds b_out.

v3: transposed attention flow. Scores are computed directly in [k, q]
layout (lhsT = k-tile variants, rhs = q), so the AV matmul consumes exp
scores without transposing the full attention matrix (the old flow spent
1088 PE transposes + LDWEIGHTS on that). Only the 384-wide W-band around
the diagonal is additionally computed in [q, k] layout, where the
softmax statistics need it: masked partial tail sums, and the diagonal
reads for the rel-v interior coefficients. Full-row statistics
(denominator D, per-k-tile sums T_kt, prefix subsets) come free as extra
output rows of the AV matmul via an augmented V operand
[v | ones | eye16 | subL | subR]  (M=113).
"""
import sys

sys.path.insert(0, "/opt/trn_rl_repo")

import numpy as np

B, S, D, H, RR, VOC = 2, 2048, 1024, 16, 64, 129
HD = 64              # head dim
NH = 4               # heads per core
N_CORES = 8
NT = S // 128        # 16 q-tiles of 128
IMW = 512            # qrel image width (clip-padded)
IMWW = 448           # exp W-zone image width (64-col front pad)
FPAD = 64            # front pad of the exp image
SCALE = 0.125        # 1/sqrt(64)
MAV = 113            # AV lhsT cols: 64 v + 1 ones + 16 eye + 16 subL + 16 subR

_cache = {}


def _regions(t):
    """W-band bounds (in k) for q-tile t."""
    i0 = 128 * t
    wlo = max(0, i0 - 128)
    whi = 256 if t == 0 else min(S, i0 + 256)
    return i0, wlo, whi


def _build():
    import concourse.bass as bass
    import concourse.mybir as mybir
    import concourse.tile as tile
    from concourse import bacc
    from contextlib import ExitStack

    F32 = mybir.dt.float32
    F32R = mybir.dt.float32r
    F16 = mybir.dt.float16
    AP = bass.AP
    AF = mybir.ActivationFunctionType
    ALU = mybir.AluOpType
    AX = mybir.AxisListType

    nc = bacc.Bacc("TRN2", target_bir_lowering=False, debug=False,
                   num_devices=N_CORES)

    # ---------------- DRAM I/O ----------------
    xT = nc.dram_tensor("xT", [D, S], F32, kind="ExternalInput").ap()
    wq = nc.dram_tensor("wq", [D, 256], F32, kind="ExternalInput").ap()
    wk = nc.dram_tensor("wk", [D, 256], F32, kind="ExternalInput").ap()
    wv = nc.dram_tensor("wv", [D, 256], F32, kind="ExternalInput").ap()
    bq = nc.dram_tensor("bq", [128, 2], F32, kind="ExternalInput").ap()
    bk = nc.dram_tensor("bk", [128, 2], F32, kind="ExternalInput").ap()
    r01 = nc.dram_tensor("r01", [128, 2], F32, kind="ExternalInput").ap()
    relk = nc.dram_tensor("relk", [128, 512], F16, kind="ExternalInput").ap()
    rvm2 = nc.dram_tensor("rvm2", [128, 64], F16, kind="ExternalInput").ap()
    rv0r = nc.dram_tensor("rv0r", [1, 64], F16, kind="ExternalInput").ap()
    rvl = nc.dram_tensor("rvl", [1, 64], F16, kind="ExternalInput").ap()
    wout = nc.dram_tensor("wout", [128, 2, 1024], F16, kind="ExternalInput").ap()
    mlm = nc.dram_tensor("mlm", [128, 512], F16, kind="ExternalInput").ap()
    mrm = nc.dram_tensor("mrm", [128, 512], F16, kind="ExternalInput").ap()
    ident = nc.dram_tensor("ident", [128, 128], F16, kind="ExternalInput").ap()
    vaug = nc.dram_tensor("vaug", [128, 16 * NH * 49], F16,
                          kind="ExternalInput").ap()
    steps = nc.dram_tensor("steps", [128, 16], F32, kind="ExternalInput").ap()
    bvbc = nc.dram_tensor("bvbc", [64, 1024], F16, kind="ExternalInput").ap()
    out = nc.dram_tensor("out", [S, D], F16, kind="ExternalOutput").ap()
    import os
    DBG = os.environ.get("KDBG", "") == "1"
    if DBG:
        dbg_av = nc.dram_tensor("dbg_av", [128, 1024], F32,
                                kind="ExternalOutput").ap()
        dbg_row = nc.dram_tensor("dbg_row", [1, 128 * 24], F16,
                                 kind="ExternalOutput").ap()
        dbg_stg = nc.dram_tensor("dbg_stg", [128, NH * 8 * 128], F16,
                                 kind="ExternalOutput").ap()
        dbg_ar = nc.dram_tensor("dbg_ar", [128, 2 * 512], F16,
                                kind="ExternalOutput").ap()
        dbg_et = nc.dram_tensor("dbg_et", [128, 4 * 512], F16,
                                kind="ExternalOutput").ap()
        DBG_G = 3

    # DRAM scratch images
    imgq_t = nc.dram_tensor("imgq", [NT * NH * 128 * IMW], F16)   # qrel pad
    imgw_t = nc.dram_tensor("imgw", [NT * NH * 128 * IMWW], F16)  # exp W-band
    statd_t = nc.dram_tensor("statd", [2 * 49 * 1024], F32)       # stat rows

    def qbase(t):
        return t * NH * 128 * IMW

    def wbase(t, h=0):
        return (t * NH + h) * 128 * IMWW

    with tile.TileContext(nc) as tc, ExitStack() as ctx:
        # ---------------- persistent pools ----------------
        pp = ctx.enter_context(tc.tile_pool(name="persist", bufs=1))
        qkT = []   # per pair: qT16, kW16, kL16, kR16  [128, S] fp16
        for pair in range(2):
            qkT.append({
                "q": pp.tile([128, S], F16, tag=f"qT{pair}", name=f"qT{pair}"),
                "W": pp.tile([128, S], F16, tag=f"kW{pair}", name=f"kW{pair}"),
                "L": pp.tile([128, S], F16, tag=f"kL{pair}", name=f"kL{pair}"),
                "R": pp.tile([128, S], F16, tag=f"kR{pair}", name=f"kR{pair}"),
            })
        # augmented V: [j, kt, h, 113] = [v_h | ones | eye16 | subL | subR]
        vA = pp.tile([128, NT, NH, MAV], F16, tag="vA", name="vA")
        relk_sb = pp.tile([128, 512], F16, tag="relk", name="relk")
        rvm2_sb = pp.tile([128, 64], F16, tag="rvm2", name="rvm2")
        rv0r_sb = pp.tile([1, 64], F16, tag="rv0r", name="rv0r")
        rvl_sb = pp.tile([1, 64], F16, tag="rvl", name="rvl")
        wout_sb = pp.tile([128, 2, 1024], F16, tag="wout", name="wout")
        bq_sb = pp.tile([128, 2], F32, tag="bq", name="bq")
        bk_sb = pp.tile([128, 2], F32, tag="bk", name="bk")
        r01_sb = pp.tile([128, 2], F32, tag="r01", name="r01")
        ml_sb = pp.tile([128, 512], F16, tag="ml", name="ml")
        mr_sb = pp.tile([128, 512], F16, tag="mr", name="mr")
        id_sb = pp.tile([128, 128], F16, tag="ident", name="ident")
        steps_sb = pp.tile([128, 2, 8], F32, tag="steps", name="steps")
        bvbc_sb = pp.tile([64, 2, 2, 256], F16, tag="bvbc", name="bvbc")
        ones1 = pp.tile([1, 128], F16, tag="ones1", name="ones1")
        zeros16 = pp.tile([128, 128], F16, tag="zeros16", name="zeros16")

        # urgent loads (phase 1 deps) on the sync queue; everything else
        # on the scalar HWDGE queue so it doesn't delay the x/W loads
        nc.sync.dma_start(bq_sb[:], bq)
        nc.sync.dma_start(bk_sb[:], bk)
        nc.sync.dma_start(r01_sb[:], r01)
        nc.scalar.dma_start(relk_sb[:], relk)
        nc.scalar.dma_start(rvm2_sb[:], rvm2)
        nc.scalar.dma_start(rv0r_sb[:], rv0r)
        nc.scalar.dma_start(rvl_sb[:], rvl)
        nc.scalar.dma_start(wout_sb[:], wout)
        nc.scalar.dma_start(ml_sb[:], mlm)
        nc.scalar.dma_start(mr_sb[:], mrm)
        nc.scalar.dma_start(id_sb[:], ident)
        nc.scalar.dma_start(vA[:, :, :, 64:MAV], vaug)
        nc.scalar.dma_start(steps_sb[:], steps)
        nc.scalar.dma_start(bvbc_sb[:], bvbc)
        nc.gpsimd.memset(ones1[:], 1.0)
        nc.gpsimd.memset(zeros16[:], 0.0)
        # zero the t=0 front pad (cols [0,64)) and t=15 back pad
        # (cols [320,448)) of the exp images so diagonal reads see 0
        for h in range(NH):
            nc.scalar.dma_start(
                AP(imgw_t, wbase(0, h), [[IMWW, 128], [1, FPAD]]),
                zeros16[:, 0:FPAD])
            nc.scalar.dma_start(
                AP(imgw_t, wbase(15, h) + FPAD + 256,
                   [[IMWW, 128], [1, 128]]),
                zeros16[:, 0:128])

        # ---------------- phase 1: projections ----------------
        with tc.tile_pool(name="p1", bufs=1) as p1, \
             tc.tile_pool(name="p1ps", bufs=2, space="PSUM") as p1ps:
            xT_sb = p1.tile([128, 8, S], F32R, tag="xT", name="xT")
            wq_sb = p1.tile([128, 8, 256], F32R, tag="wq", name="wq")
            wk_sb = p1.tile([128, 8, 256], F32R, tag="wk", name="wk")
            wv_sb = p1.tile([128, 8, 256], F32R, tag="wv", name="wv")
            xTr = xT.rearrange("(c p) s -> p c s", p=128).bitcast(F32R)
            nc.sync.dma_start(wq_sb[:], wq.rearrange("(c p) n -> p c n", p=128).bitcast(F32R))
            nc.sync.dma_start(xT_sb[:, :, 0:512], xTr[:, :, 0:512])
            nc.sync.dma_start(wk_sb[:], wk.rearrange("(c p) n -> p c n", p=128).bitcast(F32R))
            for sc in range(1, 4):
                nc.sync.dma_start(xT_sb[:, :, 512 * sc:512 * sc + 512],
                                  xTr[:, :, 512 * sc:512 * sc + 512])
            nc.sync.dma_start(wv_sb[:], wv.rearrange("(c p) n -> p c n", p=128).bitcast(F32R))

            # q, k (transposed layout [col, s]) per pair
            for pair in range(2):
                for sc in range(4):  # s-chunks of 512
                    ps_q = p1ps.tile([128, 512], F32, tag="p1q", name="p1q")
                    ps_k = p1ps.tile([128, 512], F32, tag="p1k", name="p1k")
                    for dk in range(8):
                        nc.tensor.matmul(
                            ps_q[:], wq_sb[:, dk, 128 * pair:128 * pair + 128],
                            xT_sb[:, dk, 512 * sc:512 * sc + 512],
                            start=(dk == 0), stop=(dk == 7))
                        nc.tensor.matmul(
                            ps_k[:], wk_sb[:, dk, 128 * pair:128 * pair + 128],
                            xT_sb[:, dk, 512 * sc:512 * sc + 512],
                            start=(dk == 0), stop=(dk == 7))
                    cs = slice(512 * sc, 512 * sc + 512)
                    nc.vector.tensor_scalar(
                        qkT[pair]["q"][:, cs], ps_q[:], bq_sb[:, pair:pair + 1],
                        SCALE, op0=ALU.add, op1=ALU.mult)
                    nc.vector.tensor_scalar_add(
                        qkT[pair]["W"][:, cs], ps_k[:], bk_sb[:, pair:pair + 1])
                    nc.vector.tensor_scalar_add(
                        qkT[pair]["L"][:, cs], qkT[pair]["W"][:, cs],
                        r01_sb[:, 0:1])
                    nc.vector.tensor_scalar_add(
                        qkT[pair]["R"][:, cs], qkT[pair]["W"][:, cs],
                        r01_sb[:, 1:2])

            # v (natural layout [s, col]) into vA content columns
            for st in range(NT):
                ps_v = p1ps.tile([128, 256], F32, tag="p1v", name="p1v")
                for dk in range(8):
                    nc.tensor.matmul(
                        ps_v[:], xT_sb[:, dk, 128 * st:128 * st + 128],
                        wv_sb[:, dk, :], start=(dk == 0), stop=(dk == 7))
                nc.vector.tensor_copy(
                    vA[:, st, :, 0:64],
                    ps_v[:].rearrange("p (h d) -> p h d", h=NH))

        # ---------------- phase 1.5: qrel images for all tiles ----------
        with tc.tile_pool(name="qrp", bufs=2) as qrp, \
             tc.tile_pool(name="qrps", bufs=1, space="PSUM") as qrps:
            for t in range(NT):
                qrelpad = qrp.tile([128, NH, IMW], F16, tag="qrelpad",
                                   name="qrelpad")
                for pair in range(2):
                    for h01 in range(2):
                        h = 2 * pair + h01
                        rs = slice(64 * h01, 64 * h01 + 64)
                        qr = qrps.tile([128, 512], F32, tag="qr", name="qr")
                        nc.tensor.matmul(
                            qr[:], qkT[pair]["q"][rs, 128 * t:128 * t + 128],
                            relk_sb[rs, :], start=True, stop=True)
                        if h % 2 == 0:
                            nc.vector.tensor_copy(qrelpad[:, h, :], qr[:])
                        else:
                            nc.scalar.activation(qrelpad[:, h, :], qr[:],
                                                 AF.Copy)
                nc.sync.dma_start(
                    AP(imgq_t, qbase(t),
                       [[IMW, 128], [128 * IMW, NH], [1, IMW]]),
                    qrelpad[:])

        # ---------------- phase 2 pools ----------------
        stgp = ctx.enter_context(tc.tile_pool(name="stgp", bufs=2))
        bndp = ctx.enter_context(tc.tile_pool(name="bndp", bufs=2))
        expwp = ctx.enter_context(tc.tile_pool(name="expwp", bufs=9))
        scrp = ctx.enter_context(tc.tile_pool(name="scrp", bufs=4))
        arp = ctx.enter_context(tc.tile_pool(name="arp", bufs=6))
        etp = ctx.enter_context(tc.tile_pool(name="etp", bufs=5))
        stp = ctx.enter_context(tc.tile_pool(name="stp", bufs=2))
        colp = ctx.enter_context(tc.tile_pool(name="colp", bufs=2))
        atsp = ctx.enter_context(tc.tile_pool(name="atsp", bufs=2))
        ctp = ctx.enter_context(tc.tile_pool(name="ctp", bufs=2))
        ct16p = ctx.enter_context(tc.tile_pool(name="ct16p", bufs=2))
        outp = ctx.enter_context(tc.tile_pool(name="outp", bufs=2))
        # PSUM: av 2 banks x1, qe 1 bank x4, wqps 1 bank x2 = 8 banks
        avp = ctx.enter_context(tc.tile_pool(name="avp", bufs=1, space="PSUM"))
        qep = ctx.enter_context(tc.tile_pool(name="qep", bufs=4, space="PSUM"))
        wqp = ctx.enter_context(tc.tile_pool(name="wqp", bufs=2, space="PSUM"))

        def emit_wstage(g):
            """q-layout W-band for q-tiles 2g, 2g+1: exp, tails, imgw write,
            PE transposes into staging for the transposed AV flow."""
            stg = stgp.tile([128, NH, 4, 2, 128], F16, tag="stg", name="stg")
            pkL = colp.tile([128, 8], F32, tag="pkL", name="pkL")
            pkR = colp.tile([128, 8], F32, tag="pkR", name="pkR")
            arel16s = []
            expws = {}
            # pass 1: scores + band + exp + tails (psum freed at the DVE add)
            for tq in range(2):
                t = 2 * g + tq
                i0, wlo, whi = _regions(t)
                wlen = whi - wlo
                moff = 128 if t == 0 else 0
                band4 = bndp.tile([128, NH, 384], F16, tag="band4",
                                  name="band4")
                nc.sync.dma_start(
                    band4[:, :, 0:wlen],
                    AP(imgq_t, qbase(t) + 256 - (i0 - wlo),
                       [[IMW - 1, 128], [128 * IMW, NH], [1, wlen]]))
                for pair in range(2):
                    for h01 in range(2):
                        h = 2 * pair + h01
                        u = 4 * tq + 2 * pair + h01
                        rs = slice(64 * h01, 64 * h01 + 64)
                        wqps = wqp.tile([128, 384], F32, tag="wqps",
                                        name="wqps")
                        nc.tensor.matmul(
                            wqps[:, 0:wlen],
                            qkT[pair]["q"][rs, 128 * t:128 * t + 128],
                            qkT[pair]["W"][rs, wlo:whi],
                            start=True, stop=True)
                        scW = scrp.tile([128, 384], F16, tag="scW",
                                        name="scW")
                        nc.vector.tensor_add(
                            scW[:, 0:wlen], wqps[:, 0:wlen],
                            band4[:, h, 0:wlen])
                        expw = expwp.tile([128, 384], F16, tag="expw",
                                          name="expw")
                        nc.scalar.activation(expw[:, 0:wlen], scW[:, 0:wlen],
                                             AF.Exp)
                        expws[(tq, h)] = expw
                        nc.sync.dma_start(
                            AP(imgw_t, wbase(t, h) + FPAD,
                               [[IMWW, 128], [1, wlen]]),
                            expw[:, 0:wlen])
                        # masked partial tails -> pk[:, u]
                        scr = scrp.tile([128, 384], F16, tag="scr", name="scr")
                        nc.gpsimd.tensor_tensor(
                            scr[:, 0:wlen], expw[:, 0:wlen],
                            ml_sb[:, moff:moff + wlen], op=ALU.mult)
                        nc.vector.tensor_reduce(
                            pkL[:, u:u + 1], scr[:, 0:wlen], axis=AX.X,
                            op=ALU.add)
                        scr2 = scrp.tile([128, 384], F16, tag="scr",
                                         name="scr2")
                        nc.gpsimd.tensor_tensor(
                            scr2[:, 0:wlen], expw[:, 0:wlen],
                            mr_sb[:, moff:moff + wlen], op=ALU.mult)
                        nc.vector.tensor_reduce(
                            pkR[:, u:u + 1], scr2[:, 0:wlen], axis=AX.X,
                            op=ALU.add)
                arel16 = arp.tile([128, NH, 127], F16, tag="arel16",
                                  name="arel16")
                nc.sync.dma_start(
                    arel16[:],
                    AP(imgw_t, wbase(t) + (i0 - wlo) + 1,
                       [[IMWW + 1, 128], [128 * IMWW, NH], [1, 127]]))
                arel16s.append(arel16)
            # pass 2: PE transposes into staging (exps are long done)
            for tq in range(2):
                t = 2 * g + tq
                i0, wlo, whi = _regions(t)
                kt0 = wlo // 128
                ktrel0 = kt0 - (2 * g - 1)
                nb = (whi - wlo) // 128
                for h in range(NH):
                    expw = expws[(tq, h)]
                    tp = wqp.tile([128, 384], F16, tag="wqps", name="tp")
                    for b in range(nb):
                        nc.tensor.transpose(
                            tp[:, 128 * b:128 * b + 128],
                            expw[:, 128 * b:128 * b + 128], id_sb[:])
                    nc.vector.tensor_copy(
                        stg[:, h, ktrel0:ktrel0 + nb, tq, :],
                        tp[:, 0:128 * nb].rearrange("p (b c) -> p b c", b=nb))
            return {"stg": stg, "pkL": pkL, "pkR": pkR, "arel": arel16s}

        def emit_ktloop(g, wctx):
            """Transposed scores (far zones) + exp + AV accumulation."""
            g0 = 256 * g
            stg = wctx["stg"]
            av = avp.tile([128, 2, 2, 256], F32, tag="av", name="av")
            pend_av = []   # per-chunk AV job lists, delayed behind exp
            DELAY = 2

            def emit_av_jobs(jobs):
                for kt, pair, h01, rhs in jobs:
                    h = 2 * pair + h01
                    nc.tensor.matmul(
                        av[0:MAV, pair, h01, :], vA[:, kt, h, :], rhs,
                        start=(kt == 0 and h01 == 0),
                        stop=(kt == 15), skip_group_check=True)

            def flush_av(keep=0):
                while len(pend_av) > keep:
                    emit_av_jobs(pend_av.pop(0))

            for m in range(8):
                for pair in range(2):
                    for h01 in range(2):
                        h = 2 * pair + h01
                        if m == g:
                            pend_av.append(
                                [(2 * m + s2, pair, h01,
                                  stg[:, h, 1 + s2, :, :])
                                 for s2 in range(2)])
                            flush_av(keep=DELAY)
                            continue
                        rs = slice(64 * h01, 64 * h01 + 64)
                        qx = qep.tile([128, 2, 256], F32, tag="qe", name="qx")
                        et = etp.tile([128, 2, 256], F16, tag="expT",
                                      name="et")
                        fss = []
                        for s2 in range(2):
                            kt = 2 * m + s2
                            var = "L" if kt < 2 * g else "R"
                            if kt == 2 * g - 1:
                                fs = slice(128, 256)   # far cols of group
                            elif kt == 2 * g + 2:
                                fs = slice(0, 128)
                            else:
                                fs = slice(0, 256)
                            fss.append(fs)
                            nc.tensor.matmul(
                                qx[:, s2, fs],
                                qkT[pair][var][rs, 128 * kt:128 * kt + 128],
                                qkT[pair]["q"][rs, g0 + fs.start:g0 + fs.stop],
                                start=True, stop=True)
                        flush_av(keep=DELAY)
                        if fss[0] == slice(0, 256) and fss[1] == slice(0, 256):
                            nc.scalar.activation(et[:], qx[:], AF.Exp)
                        else:
                            for s2 in range(2):
                                nc.scalar.activation(
                                    et[:, s2, fss[s2]], qx[:, s2, fss[s2]],
                                    AF.Exp)
                        for s2 in range(2):
                            kt = 2 * m + s2
                            if kt == 2 * g - 1:
                                nc.vector.tensor_copy(
                                    et[:, s2, 0:128], stg[:, h, 0, 0, :])
                            elif kt == 2 * g + 2:
                                nc.vector.tensor_copy(
                                    et[:, s2, 128:256], stg[:, h, 3, 1, :])
                        if DBG and g == DBG_G and m == 0:
                            nc.sync.dma_start(
                                dbg_et[:, 512 * h:512 * h + 512],
                                et.rearrange("p a b -> p (a b)"))
                        pend_av.append([(2 * m + s2, pair, h01, et[:, s2, :])
                                        for s2 in range(2)])
            flush_av()
            return av

        def emit_finish_a(g, wctx, av):
            """Evacuate av to SBUF (frees the PSUM bank) and run the
            stats flips + sL/sR assembly (DVE + DMA only, no tensor)."""
            pkL, pkR = wctx["pkL"], wctx["pkR"]
            # all av rows to SBUF; rows 64.. also to DRAM for the flips
            stats = stp.tile([128, 2, 2, 256], F32, tag="stats", name="stats")
            nc.vector.tensor_copy(stats[0:MAV], av[0:MAV])
            sb0 = (g & 1) * 49 * 1024
            nc.sync.dma_start(
                AP(statd_t, sb0, [[1024, 49], [1, 1024]]), stats[64:MAV])
            Dcol = colp.tile([128, 8], F32, tag="Dcol", name="Dcol")
            TcolL = colp.tile([128, 8], F32, tag="TcolL", name="TcolL")
            TcolR = colp.tile([128, 8], F32, tag="TcolR", name="TcolR")
            subL8 = colp.tile([128, 8], F32, tag="subL8", name="subL8")
            subR8 = colp.tile([128, 8], F32, tag="subR8", name="subR8")
            for tq in range(2):
                t = 2 * g + tq
                # statd row r: 0=D, 1+kt=T_kt, 17+t=subL_t, 33+t=subR_t
                rL = t if t > 0 else 0        # T_{t-1}; junk for t=0
                rRr = t + 2 if t < 15 else 0  # T_{t+1}; junk for t=15
                for dst, r in ((Dcol, 0), (TcolL, rL), (TcolR, rRr),
                               (subL8, 17 + t), (subR8, 33 + t)):
                    nc.sync.dma_start(
                        dst[:, 4 * tq:4 * tq + 4].rearrange(
                            "p (a b) -> p a b", a=2),
                        AP(statd_t, sb0 + r * 1024 + 128 * tq,
                           [[1, 128], [512, 2], [256, 2]]))
            sL8 = colp.tile([128, 8], F32, tag="sL8", name="sL8")
            sR8 = colp.tile([128, 8], F32, tag="sR8", name="sR8")
            nc.vector.tensor_tensor(sL8[:], TcolL[:], steps_sb[:, 0, :],
                                    op=ALU.mult)
            nc.vector.tensor_add(sL8[:], sL8[:], subL8[:])
            nc.vector.tensor_add(sL8[:], sL8[:], pkL[:])
            nc.vector.tensor_tensor(sR8[:], TcolR[:], steps_sb[:, 1, :],
                                    op=ALU.mult)
            nc.vector.tensor_add(sR8[:], sR8[:], subR8[:])
            nc.vector.tensor_add(sR8[:], sR8[:], pkR[:])
            if g == 0:   # t=0 has no T_{t-1} term
                nc.vector.tensor_tensor(sL8[:, 0:4], subL8[:, 0:4],
                                        pkL[:, 0:4], op=ALU.add)
            if g == 7:   # t=15 has no T_{t+1} term
                nc.vector.tensor_tensor(sR8[:, 4:8], subR8[:, 4:8],
                                        pkR[:, 4:8], op=ALU.add)
            rec = colp.tile([128, 8], F32, tag="rec", name="rec")
            nc.vector.reciprocal(rec[:], Dcol[:])
            # pack to fp16 rows: chans [rec | sL | sR], each (pr, h01, tq)
            pkout = colp.tile([128, 24], F16, tag="pkout", name="pkout")
            for blk, src in ((0, rec), (8, sL8), (16, sR8)):
                nc.vector.tensor_copy(
                    pkout[:, blk:blk + 8].rearrange(
                        "p (a b c) -> p c a b", a=2, b=2),
                    src[:].rearrange("p (c a b) -> p c a b", c=2, a=2))
            rowout = colp.tile([1, 128, 24], F16, tag="rowout", name="rowout")
            nc.sync.dma_start(rowout.rearrange("o p c -> o (p c)"), pkout[:])
            if DBG and g == DBG_G:
                nc.sync.dma_start(dbg_av,
                                  stats.rearrange("p a b c -> p (a b c)"))
                nc.sync.dma_start(dbg_row, rowout.rearrange("o p c -> o (p c)"))
                nc.sync.dma_start(
                    dbg_stg,
                    wctx["stg"].rearrange("p a b c d -> p (a b c d)"))
            return {"stats": stats, "rowout": rowout}

        def emit_finish_b(g, wctx, fctx):
            """rel-v matmuls, normalization, output projection."""
            arel16s = wctx["arel"]
            stats = fctx["stats"]
            rowout = fctx["rowout"]
            ct16 = {}
            for pair in range(2):
                # rel-v interior via transposed diagonal coefficients
                atp = wqp.tile([127, 2, 2, 128], F16, tag="wqps", name="atp")
                for h01 in range(2):
                    for tq in range(2):
                        nc.tensor.transpose(
                            atp[0:127, h01, tq, :],
                            arel16s[tq][:, 2 * pair + h01, 0:127], id_sb[:])
                arelTs = atsp.tile([127, 2, 2, 128], F16, tag="arelTs",
                                   name="arelTs")
                nc.vector.tensor_copy(arelTs[0:127], atp[0:127])
                if DBG and g == DBG_G:
                    nc.sync.dma_start(
                        dbg_ar[0:127, 512 * pair:512 * pair + 512],
                        arelTs.rearrange("p a b c -> p (a b c)"))
                relps = qep.tile([64, 2, 256], F32, tag="qe", name="relps")
                nc.tensor.matmul(
                    relps[:], rvm2_sb[0:127, :],
                    arelTs[0:127].rearrange("p a b c -> p (a b c)"),
                    start=True, stop=False)
                nc.tensor.matmul(
                    relps[:], rv0r_sb[:],
                    rowout[0:1, :, 8 + 4 * pair:12 + 4 * pair].rearrange(
                        "o p (a b) -> o a b p", a=2),
                    start=False, stop=False)
                nc.tensor.matmul(
                    relps[:], rvl_sb[:],
                    rowout[0:1, :, 16 + 4 * pair:20 + 4 * pair].rearrange(
                        "o p (a b) -> o a b p", a=2),
                    start=False, stop=True)
                bcps = qep.tile([64, 2, 256], F32, tag="qe", name="bcps")
                nc.tensor.matmul(
                    bcps[:], ones1[0:1, 0:64],
                    rowout[0:1, :, 4 * pair:4 * pair + 4].rearrange(
                        "o p (a b) -> o a b p", a=2),
                    start=True, stop=True)
                rbc = ctp.tile([64, 2, 256], F16, tag="rbc", name="rbc")
                nc.vector.tensor_copy(rbc[:], bcps[:])
                ctmp = ctp.tile([64, 2, 256], F16, tag="ctmp", name="ctmp")
                nc.vector.tensor_add(ctmp[:], stats[0:64, pair, :, :],
                                     relps[:])
                nc.vector.tensor_tensor(ctmp[:], ctmp[:], rbc[:],
                                        op=ALU.mult)
                nc.vector.tensor_add(ctmp[:], ctmp[:], bvbc_sb[:, pair, :, :])
                ct = ct16p.tile([128, 256], F16, tag=f"ct{pair}",
                                name=f"ct{pair}")
                nc.vector.tensor_copy(ct[0:64, :], ctmp[:, 0, :])
                nc.sync.dma_start(ct[64:128, :], ctmp[:, 1, :])
                ct16[pair] = ct

            for tq in range(2):
                out_sb = outp.tile([128, 1024], F16, tag="out_sb",
                                   name="out_sb")
                for nch in range(2):
                    op_ps = qep.tile([128, 512], F32, tag="qe", name="op_ps")
                    for pair in range(2):
                        nc.tensor.matmul(
                            op_ps[:], ct16[pair][:, 128 * tq:128 * tq + 128],
                            wout_sb[:, pair, 512 * nch:512 * nch + 512],
                            start=(pair == 0), stop=(pair == 1))
                    nc.vector.tensor_copy(
                        out_sb[:, 512 * nch:512 * nch + 512], op_ps[:])
                r0_ = 256 * g + 128 * tq
                nc.sync.dma_start(out[r0_:r0_ + 128, :], out_sb[:])

        # ---------------- phase 2: software-pipelined groups -------------
        # finish_a(g) (DVE/DMA stats evacuation) runs right after
        # ktloop(g); finish_b(g) (tensor) trails ktloop(g+1) so its
        # serial stats chain hides under the next group's tensor work
        wctx = emit_wstage(0)
        pend = None   # (g, wctx, fctx) awaiting finish_b
        for g in range(8):
            av = emit_ktloop(g, wctx)
            fctx = emit_finish_a(g, wctx, av)
            cur = (g, wctx, fctx)
            if g < 7:
                nxt = emit_wstage(g + 1)
            if pend is not None:
                emit_finish_b(*pend)
            pend = cur
            if g < 7:
                wctx = nxt
        emit_finish_b(*pend)

    nc.compile()
    return nc


def get_nc():
    if "nc" not in _cache:
        _cache["nc"] = _build()
    return _cache["nc"]


def shard_inputs(inputs):
    """Build per-core input maps from full inputs (layout prep only)."""
    x = np.asarray(inputs["x"], np.float32)
    W_qkv = np.asarray(inputs["W_qkv"], np.float32)
    b_qkv = np.asarray(inputs["b_qkv"], np.float32)
    W_out = np.asarray(inputs["W_out"], np.float32)
    rk = np.asarray(inputs["rel_emb_k"], np.float32)
    rv = np.asarray(inputs["rel_emb_v"], np.float32)

    Wq, Wk, Wv = W_qkv[:, 0:D], W_qkv[:, D:2 * D], W_qkv[:, 2 * D:3 * D]
    bqf, bkf, bvf = b_qkv[0:D], b_qkv[D:2 * D], b_qkv[2 * D:3 * D]

    cidx = np.clip(np.arange(512) - 256, -64, 64) + 64   # [512] vocab index
    relk_pad = rk.T[:, cidx].astype(np.float16)           # [64, 512]
    relk_host = np.concatenate([relk_pad, relk_pad], axis=0)  # [128, 512]
    rvm2_host = np.zeros((128, 64), np.float16)
    rvm2_host[0:127] = rv[1:128].astype(np.float16)
    rv0r_host = rv[0:1].astype(np.float16)
    rvl_host = rv[128:129].astype(np.float16)
    r0 = np.tile(rk[0], 2).reshape(128, 1)
    r1 = np.tile(rk[128], 2).reshape(128, 1)
    r01_host = np.concatenate([r0, r1], 1).astype(np.float32)

    # tail masks [128, 512] master: middle tiles slice [0:wlen],
    # t=0 slices [128:128+wlen]
    jj = np.arange(512)[None, :]
    ppi = np.arange(128)[:, None]
    ml_host = ((jj <= ppi + 64) &
               (jj >= 128 * (ppi >= 64))).astype(np.float16)
    mr_host = ((jj >= ppi + 192) &
               (jj <= 255 + 128 * (ppi >= 64))).astype(np.float16)
    ident_host = np.eye(128, dtype=np.float16)

    # augmented-V constant columns [kt, h, 49]:
    # col 0: ones (D); 1..16: eye16 (T_kt); 17..32: subL; 33..48: subR
    aug = np.zeros((16, 49), np.float16)
    aug[:, 0] = 1.0
    for kt in range(16):
        aug[kt, 1 + kt] = 1.0
        for t in range(16):
            if kt <= t - 2:
                aug[kt, 17 + t] = 1.0
            if kt >= t + 2:
                aug[kt, 33 + t] = 1.0
    vaug_host = np.broadcast_to(
        np.repeat(aug[:, None, :], NH, axis=1).reshape(1, -1),
        (128, 16 * NH * 49)).astype(np.float16).copy()

    # step gates for the boundary T_kt term: L uses i_loc>=64, R uses <64
    iloc = np.arange(128)[:, None]
    st_hi = (iloc >= 64).astype(np.float32)
    st_lo = (iloc < 64).astype(np.float32)
    steps_host = np.concatenate(
        [np.repeat(st_hi, 8, 1), np.repeat(st_lo, 8, 1)], 1)

    in_maps = []
    for c in range(N_CORES):
        b, gg = c // 4, c % 4
        cols = slice(256 * gg, 256 * gg + 256)
        bvc = bvf[cols].reshape(2, 2, 64)          # [pair, h01, hd]
        bvbc_host = np.broadcast_to(
            bvc.transpose(2, 0, 1)[:, :, :, None],
            (64, 2, 2, 256)).reshape(64, 1024).astype(np.float16).copy()
        m = {
            "xT": np.ascontiguousarray(x[b].T),
            "wq": np.ascontiguousarray(Wq[:, cols]),
            "wk": np.ascontiguousarray(Wk[:, cols]),
            "wv": np.ascontiguousarray(Wv[:, cols]),
            "bq": np.ascontiguousarray(bqf[cols].reshape(2, 128).T),
            "bk": np.ascontiguousarray(bkf[cols].reshape(2, 128).T),
            "r01": r01_host,
            "relk": relk_host,
            "rvm2": rvm2_host,
            "rv0r": rv0r_host,
            "rvl": rvl_host,
            "wout": np.ascontiguousarray(
                W_out[cols].reshape(2, 128, 1024).transpose(1, 0, 2)
            ).astype(np.float16),
            "mlm": ml_host,
            "mrm": mr_host,
            "ident": ident_host,
            "vaug": vaug_host,
            "steps": steps_host,
            "bvbc": bvbc_host,
        }
        in_maps.append(m)
    return in_maps


def unshard_outputs(results, inputs):
    b_out = np.asarray(inputs["b_out"], np.float32)
    out = np.zeros((B, S, D), np.float32)
    for c in range(N_CORES):
        out[c // 4] += results[c]["out"].astype(np.float32)
    out += b_out[None, None, :]
    return out


def kernel(**inputs):
    from concourse import bass_utils
    nc = get_nc()
    in_maps = shard_inputs(inputs)
    res = bass_utils.run_bass_kernel_spmd(nc, in_maps, list(range(N_CORES)))
    return unshard_outputs(res.results, inputs)


if __name__ == "__main__":
    rng = np.random.default_rng(0)
    demo = {
        "x": rng.standard_normal((B, S, D)).astype(np.float32),
        "W_qkv": (rng.standard_normal((D, 3 * D)) * 0.02).astype(np.float32),
        "b_qkv": np.zeros(3 * D, np.float32),
        "W_out": (rng.standard_normal((D, D)) * 0.02).astype(np.float32),
        "b_out": np.zeros(D, np.float32),
        "rel_emb_k": (rng.standard_normal((VOC, HD)) * 0.02).astype(np.float32),
        "rel_emb_v": (rng.standard_normal((VOC, HD)) * 0.02).astype(np.float32),
    }
    o = kernel(**demo)
    print(o.shape, float(np.abs(o).max()))


# revision 51
# speedup vs baseline: 1.1516x; 1.0047x over previous
"""Trainium2 Bass kernel for MultiHeadedSelfAttention with Shaw relative
position embeddings (clipped, R=64), sharded over 8 NeuronCores.

Sharding: core c handles batch b = c//4 and head group g = c%4 (4 heads).
Each core computes a partial output  ctx_g @ W_out[256g:256g+256]  for its
batch; the host sums the 4 partials per batch and adds b_out.

v3: transposed attention flow. Scores are computed directly in [k, q]
layout (lhsT = k-tile variants, rhs = q), so the AV matmul consumes exp
scores without transposing the full attention matrix (the old flow spent
1088 PE transposes + LDWEIGHTS on that). Only the 384-wide W-band around
the diagonal is additionally computed in [q, k] layout, where the
softmax statistics need it: masked partial tail sums, and the diagonal
reads for the rel-v interior coefficients. Full-row statistics
(denominator D, per-k-tile sums T_kt, prefix subsets) come free as extra
output rows of the AV matmul via an augmented V operand
[v | ones | eye16 | subL | subR]  (M=113).
"""
import sys

sys.path.insert(0, "/opt/trn_rl_repo")

import numpy as np

B, S, D, H, RR, VOC = 2, 2048, 1024, 16, 64, 129
HD = 64              # head dim
NH = 4               # heads per core
N_CORES = 8
NT = S // 128        # 16 q-tiles of 128
IMW = 512            # qrel image width (clip-padded)
IMWW = 448           # exp W-zone image width (64-col front pad)
FPAD = 64            # front pad of the exp image
SCALE = 0.125        # 1/sqrt(64)
MAV = 116            # 64 v | ones | eye16 | subL16 | subR16 | pad3 (NH*MAV %16==0)

_cache = {}


def _regions(t):
    """W-band bounds (in k) for q-tile t."""
    i0 = 128 * t
    wlo = max(0, i0 - 128)
    whi = 256 if t == 0 else min(S, i0 + 256)
    return i0, wlo, whi


def _build():
    import concourse.bass as bass
    import concourse.mybir as mybir
    import concourse.tile as tile
    from concourse import bacc
    from contextlib import ExitStack

    F32 = mybir.dt.float32
    F32R = mybir.dt.float32r
    F16 = mybir.dt.float16
    F8 = mybir.dt.float8e4
    DR = mybir.MatmulPerfMode.DoubleRow
    AP = bass.AP
    AF = mybir.ActivationFunctionType
    ALU = mybir.AluOpType
    AX = mybir.AxisListType

    nc = bacc.Bacc("TRN2", target_bir_lowering=False, debug=False,
                   num_devices=N_CORES)

    # ---------------- DRAM I/O ----------------
    xT = nc.dram_tensor("xT", [D, S], F32, kind="ExternalInput").ap()
    wq = nc.dram_tensor("wq", [D, 256], F32, kind="ExternalInput").ap()
    wk = nc.dram_tensor("wk", [D, 256], F32, kind="ExternalInput").ap()
    wv = nc.dram_tensor("wv", [D, 256], F32, kind="ExternalInput").ap()
    bq = nc.dram_tensor("bq", [128, 2], F32, kind="ExternalInput").ap()
    bk = nc.dram_tensor("bk", [128, 2], F32, kind="ExternalInput").ap()
    r01 = nc.dram_tensor("r01", [128, 2], F32, kind="ExternalInput").ap()
    relk = nc.dram_tensor("relk", [128, 512], F16, kind="ExternalInput").ap()
    rvm2 = nc.dram_tensor("rvm2", [128, 64], F16, kind="ExternalInput").ap()
    rv0r = nc.dram_tensor("rv0r", [1, 64], F16, kind="ExternalInput").ap()
    rvl = nc.dram_tensor("rvl", [1, 64], F16, kind="ExternalInput").ap()
    wout = nc.dram_tensor("wout", [128, 2, 1024], F16, kind="ExternalInput").ap()
    mlm = nc.dram_tensor("mlm", [128, 512], F16, kind="ExternalInput").ap()
    mrm = nc.dram_tensor("mrm", [128, 512], F16, kind="ExternalInput").ap()
    ident = nc.dram_tensor("ident", [128, 128], F16, kind="ExternalInput").ap()
    vaug = nc.dram_tensor("vaug", [128, 16 * NH * 52], F16,
                          kind="ExternalInput").ap()
    steps = nc.dram_tensor("steps", [128, 16], F32, kind="ExternalInput").ap()
    bvbc = nc.dram_tensor("bvbc", [64, 1024], F16, kind="ExternalInput").ap()
    out = nc.dram_tensor("out", [S, D], F16, kind="ExternalOutput").ap()
    import os
    DBG = os.environ.get("KDBG", "") == "1"
    if DBG:
        dbg_av = nc.dram_tensor("dbg_av", [128, 1024], F32,
                                kind="ExternalOutput").ap()
        dbg_row = nc.dram_tensor("dbg_row", [1, 128 * 24], F16,
                                 kind="ExternalOutput").ap()
        dbg_stg = nc.dram_tensor("dbg_stg", [128, NH * 8 * 128], F16,
                                 kind="ExternalOutput").ap()
        dbg_ar = nc.dram_tensor("dbg_ar", [128, 2 * 512], F16,
                                kind="ExternalOutput").ap()
        dbg_et = nc.dram_tensor("dbg_et", [128, 4 * 512], F16,
                                kind="ExternalOutput").ap()
        DBG_G = 3

    # DRAM scratch images
    imgq_t = nc.dram_tensor("imgq", [NT * NH * 128 * IMW], F16)   # qrel pad
    imgw_t = nc.dram_tensor("imgw", [NT * NH * 128 * IMWW], F16)  # exp W-band
    statd_t = nc.dram_tensor("statd", [2 * 49 * 1024], F32)       # stat rows

    def qbase(t):
        return t * NH * 128 * IMW

    def wbase(t, h=0):
        return (t * NH + h) * 128 * IMWW

    with tile.TileContext(nc) as tc, ExitStack() as ctx:
        # ---------------- persistent pools ----------------
        pp = ctx.enter_context(tc.tile_pool(name="persist", bufs=1))
        qkT = []   # per pair: qT16, kW16, kL16, kR16  [128, S] fp16
        for pair in range(2):
            qkT.append({
                "q": pp.tile([128, S], F16, tag=f"qT{pair}", name=f"qT{pair}"),
                "W": pp.tile([128, S], F16, tag=f"kW{pair}", name=f"kW{pair}"),
                "L": pp.tile([128, S], F16, tag=f"kL{pair}", name=f"kL{pair}"),
                "R": pp.tile([128, S], F16, tag=f"kR{pair}", name=f"kR{pair}"),
            })
        # augmented V: [j, kt, h, 113] = [v_h | ones | eye16 | subL | subR]
        vA = pp.tile([128, NT, NH, MAV], F16, tag="vA", name="vA")
        relk_sb = pp.tile([128, 512], F16, tag="relk", name="relk")
        rvm2_sb = pp.tile([128, 64], F16, tag="rvm2", name="rvm2")
        rv0r_sb = pp.tile([1, 64], F16, tag="rv0r", name="rv0r")
        rvl_sb = pp.tile([1, 64], F16, tag="rvl", name="rvl")
        wout_sb = pp.tile([128, 2, 1024], F16, tag="wout", name="wout")
        bq_sb = pp.tile([128, 2], F32, tag="bq", name="bq")
        bk_sb = pp.tile([128, 2], F32, tag="bk", name="bk")
        r01_sb = pp.tile([128, 2], F32, tag="r01", name="r01")
        ml_sb = pp.tile([128, 512], F16, tag="ml", name="ml")
        mr_sb = pp.tile([128, 512], F16, tag="mr", name="mr")
        id_sb = pp.tile([128, 128], F16, tag="ident", name="ident")
        steps_sb = pp.tile([128, 2, 8], F32, tag="steps", name="steps")
        bvbc_sb = pp.tile([64, 2, 2, 256], F16, tag="bvbc", name="bvbc")
        ones1 = pp.tile([1, 128], F16, tag="ones1", name="ones1")
        zeros16 = pp.tile([128, 128], F16, tag="zeros16", name="zeros16")
        zero32 = pp.tile([128, 1], F32, tag="zero32", name="zero32")

        # urgent loads (phase 1 deps) on the sync queue; everything else
        # on the scalar HWDGE queue so it doesn't delay the x/W loads
        nc.sync.dma_start(bq_sb[:], bq)
        nc.sync.dma_start(bk_sb[:], bk)
        nc.sync.dma_start(r01_sb[:], r01)
        nc.scalar.dma_start(relk_sb[:], relk)
        nc.scalar.dma_start(rvm2_sb[:], rvm2)
        nc.scalar.dma_start(rv0r_sb[:], rv0r)
        nc.scalar.dma_start(rvl_sb[:], rvl)
        nc.scalar.dma_start(wout_sb[:], wout)
        nc.scalar.dma_start(ml_sb[:], mlm)
        nc.scalar.dma_start(mr_sb[:], mrm)
        nc.scalar.dma_start(id_sb[:], ident)
        nc.scalar.dma_start(vA[:, :, :, 64:MAV], vaug)
        nc.scalar.dma_start(steps_sb[:], steps)
        nc.scalar.dma_start(bvbc_sb[:], bvbc)
        nc.gpsimd.memset(ones1[:], 1.0)
        nc.gpsimd.memset(zeros16[:], 0.0)
        nc.gpsimd.memset(zero32[:], 0.0)
        # zero the t=0 front pad (cols [0,64)) and t=15 back pad
        # (cols [320,448)) of the exp images so diagonal reads see 0
        for h in range(NH):
            nc.scalar.dma_start(
                AP(imgw_t, wbase(0, h), [[IMWW, 128], [1, FPAD]]),
                zeros16[:, 0:FPAD])
            nc.scalar.dma_start(
                AP(imgw_t, wbase(15, h) + FPAD + 256,
                   [[IMWW, 128], [1, 128]]),
                zeros16[:, 0:128])

        # ---------------- phase 1: projections ----------------
        with tc.tile_pool(name="p1", bufs=1) as p1, \
             tc.tile_pool(name="p1ps", bufs=2, space="PSUM") as p1ps:
            xT_sb = p1.tile([128, 8, S], F32R, tag="xT", name="xT")
            wq_sb = p1.tile([128, 8, 256], F32R, tag="wq", name="wq")
            wk_sb = p1.tile([128, 8, 256], F32R, tag="wk", name="wk")
            wv_sb = p1.tile([128, 8, 256], F32R, tag="wv", name="wv")
            xTr = xT.rearrange("(c p) s -> p c s", p=128).bitcast(F32R)
            nc.sync.dma_start(wq_sb[:], wq.rearrange("(c p) n -> p c n", p=128).bitcast(F32R))
            nc.sync.dma_start(xT_sb[:, :, 0:512], xTr[:, :, 0:512])
            nc.sync.dma_start(wk_sb[:], wk.rearrange("(c p) n -> p c n", p=128).bitcast(F32R))
            for sc in range(1, 4):
                nc.sync.dma_start(xT_sb[:, :, 512 * sc:512 * sc + 512],
                                  xTr[:, :, 512 * sc:512 * sc + 512])
            nc.sync.dma_start(wv_sb[:], wv.rearrange("(c p) n -> p c n", p=128).bitcast(F32R))

            # q, k (transposed layout [col, s]) per pair
            for pair in range(2):
                for sc in range(4):  # s-chunks of 512
                    ps_q = p1ps.tile([128, 512], F32, tag="p1q", name="p1q")
                    ps_k = p1ps.tile([128, 512], F32, tag="p1k", name="p1k")
                    for dk in range(8):
                        nc.tensor.matmul(
                            ps_q[:], wq_sb[:, dk, 128 * pair:128 * pair + 128],
                            xT_sb[:, dk, 512 * sc:512 * sc + 512],
                            start=(dk == 0), stop=(dk == 7))
                        nc.tensor.matmul(
                            ps_k[:], wk_sb[:, dk, 128 * pair:128 * pair + 128],
                            xT_sb[:, dk, 512 * sc:512 * sc + 512],
                            start=(dk == 0), stop=(dk == 7))
                    cs = slice(512 * sc, 512 * sc + 512)
                    nc.vector.tensor_scalar(
                        qkT[pair]["q"][:, cs], ps_q[:], bq_sb[:, pair:pair + 1],
                        SCALE, op0=ALU.add, op1=ALU.mult)
                    nc.vector.tensor_scalar_add(
                        qkT[pair]["W"][:, cs], ps_k[:], bk_sb[:, pair:pair + 1])
                    nc.vector.tensor_scalar_add(
                        qkT[pair]["L"][:, cs], qkT[pair]["W"][:, cs],
                        r01_sb[:, 0:1])
                    nc.vector.tensor_scalar_add(
                        qkT[pair]["R"][:, cs], qkT[pair]["W"][:, cs],
                        r01_sb[:, 1:2])

            # v (natural layout [s, col]) into vA content columns
            for st in range(NT):
                ps_v = p1ps.tile([128, 256], F32, tag="p1v", name="p1v")
                for dk in range(8):
                    nc.tensor.matmul(
                        ps_v[:], xT_sb[:, dk, 128 * st:128 * st + 128],
                        wv_sb[:, dk, :], start=(dk == 0), stop=(dk == 7))
                nc.vector.tensor_copy(
                    vA[:, st, :, 0:64],
                    ps_v[:].rearrange("p (h d) -> p h d", h=NH))

        # ---------------- phase 1.5: qrel images for all tiles ----------
        with tc.tile_pool(name="qrp", bufs=2) as qrp, \
             tc.tile_pool(name="qrps", bufs=1, space="PSUM") as qrps:
            for t in range(NT):
                qrelpad = qrp.tile([128, NH, IMW], F16, tag="qrelpad",
                                   name="qrelpad")
                for pair in range(2):
                    for h01 in range(2):
                        h = 2 * pair + h01
                        rs = slice(64 * h01, 64 * h01 + 64)
                        qr = qrps.tile([128, 512], F32, tag="qr", name="qr")
                        nc.tensor.matmul(
                            qr[:], qkT[pair]["q"][rs, 128 * t:128 * t + 128],
                            relk_sb[rs, :], start=True, stop=True)
                        if h % 2 == 0:
                            nc.vector.tensor_copy(qrelpad[:, h, :], qr[:])
                        else:
                            nc.scalar.activation(qrelpad[:, h, :], qr[:],
                                                 AF.Copy)
                nc.sync.dma_start(
                    AP(imgq_t, qbase(t),
                       [[IMW, 128], [128 * IMW, NH], [1, IMW]]),
                    qrelpad[:])

        # ---------------- phase 2 pools ----------------
        stgp = ctx.enter_context(tc.tile_pool(name="stgp", bufs=2))
        bndp = ctx.enter_context(tc.tile_pool(name="bndp", bufs=2))
        expwp = ctx.enter_context(tc.tile_pool(name="expwp", bufs=9))
        scrp = ctx.enter_context(tc.tile_pool(name="scrp", bufs=4))
        arp = ctx.enter_context(tc.tile_pool(name="arp", bufs=6))
        etp = ctx.enter_context(tc.tile_pool(name="etp", bufs=5))
        stp = ctx.enter_context(tc.tile_pool(name="stp", bufs=2))
        colp = ctx.enter_context(tc.tile_pool(name="colp", bufs=2))
        atsp = ctx.enter_context(tc.tile_pool(name="atsp", bufs=2))
        ctp = ctx.enter_context(tc.tile_pool(name="ctp", bufs=2))
        ct16p = ctx.enter_context(tc.tile_pool(name="ct16p", bufs=2))
        outp = ctx.enter_context(tc.tile_pool(name="outp", bufs=2))
        # PSUM: av 2 banks x1, qe 1 bank x4, wqps 1 bank x2 = 8 banks
        avp = ctx.enter_context(tc.tile_pool(name="avp", bufs=1, space="PSUM"))
        qep = ctx.enter_context(tc.tile_pool(name="qep", bufs=4, space="PSUM"))
        wqp = ctx.enter_context(tc.tile_pool(name="wqp", bufs=2, space="PSUM"))

        def emit_wstage(g):
            """q-layout W-band for q-tiles 2g, 2g+1: exp, tails, imgw write,
            PE transposes into staging for the transposed AV flow."""
            stg = stgp.tile([128, NH, 4, 2, 128], F16, tag="stg", name="stg")
            pkL = colp.tile([128, 8], F32, tag="pkL", name="pkL")
            pkR = colp.tile([128, 8], F32, tag="pkR", name="pkR")
            arel16s = []
            expws = {}
            # pass 1: scores + band + exp + tails (psum freed at the DVE add)
            for tq in range(2):
                t = 2 * g + tq
                i0, wlo, whi = _regions(t)
                wlen = whi - wlo
                moff = 128 if t == 0 else 0
                band4 = bndp.tile([128, NH, 384], F16, tag="band4",
                                  name="band4")
                nc.sync.dma_start(
                    band4[:, :, 0:wlen],
                    AP(imgq_t, qbase(t) + 256 - (i0 - wlo),
                       [[IMW - 1, 128], [128 * IMW, NH], [1, wlen]]))
                for pair in range(2):
                    for h01 in range(2):
                        h = 2 * pair + h01
                        u = 4 * tq + 2 * pair + h01
                        rs = slice(64 * h01, 64 * h01 + 64)
                        wqps = wqp.tile([128, 384], F32, tag="wqps",
                                        name="wqps")
                        nc.tensor.matmul(
                            wqps[:, 0:wlen],
                            qkT[pair]["q"][rs, 128 * t:128 * t + 128],
                            qkT[pair]["W"][rs, wlo:whi],
                            start=True, stop=True)
                        scW = scrp.tile([128, 384], F16, tag="scW",
                                        name="scW")
                        nc.vector.tensor_add(
                            scW[:, 0:wlen], wqps[:, 0:wlen],
                            band4[:, h, 0:wlen])
                        expw = expwp.tile([128, 384], F16, tag="expw",
                                          name="expw")
                        nc.scalar.activation(expw[:, 0:wlen], scW[:, 0:wlen],
                                             AF.Exp)
                        expws[(tq, h)] = expw
                        nc.sync.dma_start(
                            AP(imgw_t, wbase(t, h) + FPAD,
                               [[IMWW, 128], [1, wlen]]),
                            expw[:, 0:wlen])
                        # masked partial tails -> pk[:, u]
                        scr = scrp.tile([128, 384], F16, tag="scr", name="scr")
                        nc.gpsimd.tensor_tensor(
                            scr[:, 0:wlen], expw[:, 0:wlen],
                            ml_sb[:, moff:moff + wlen], op=ALU.mult)
                        nc.vector.tensor_reduce(
                            pkL[:, u:u + 1], scr[:, 0:wlen], axis=AX.X,
                            op=ALU.add)
                        scr2 = scrp.tile([128, 384], F16, tag="scr",
                                         name="scr2")
                        nc.gpsimd.tensor_tensor(
                            scr2[:, 0:wlen], expw[:, 0:wlen],
                            mr_sb[:, moff:moff + wlen], op=ALU.mult)
                        nc.vector.tensor_reduce(
                            pkR[:, u:u + 1], scr2[:, 0:wlen], axis=AX.X,
                            op=ALU.add)
                arel16 = arp.tile([128, NH, 127], F16, tag="arel16",
                                  name="arel16")
                nc.sync.dma_start(
                    arel16[:],
                    AP(imgw_t, wbase(t) + (i0 - wlo) + 1,
                       [[IMWW + 1, 128], [128 * IMWW, NH], [1, 127]]))
                arel16s.append(arel16)
            # pass 2: PE transposes into staging (exps are long done)
            for tq in range(2):
                t = 2 * g + tq
                i0, wlo, whi = _regions(t)
                kt0 = wlo // 128
                ktrel0 = kt0 - (2 * g - 1)
                nb = (whi - wlo) // 128
                for h in range(NH):
                    expw = expws[(tq, h)]
                    tp = wqp.tile([128, 384], F16, tag="wqps", name="tp")
                    for b in range(nb):
                        nc.tensor.transpose(
                            tp[:, 128 * b:128 * b + 128],
                            expw[:, 128 * b:128 * b + 128], id_sb[:])
                    nc.vector.tensor_copy(
                        stg[:, h, ktrel0:ktrel0 + nb, tq, :],
                        tp[:, 0:128 * nb].rearrange("p (b c) -> p b c", b=nb))
            return {"stg": stg, "pkL": pkL, "pkR": pkR, "arel": arel16s}

        def emit_ktloop(g, wctx):
            """Transposed scores (far zones) + exp + AV accumulation."""
            g0 = 256 * g
            stg = wctx["stg"]
            av = avp.tile([128, 2, 2, 256], F32, tag="av", name="av")
            pend_av = []   # per-chunk AV job lists, delayed behind exp
            DELAY = 2

            def emit_av_jobs(jobs):
                for kt, pair, h01, rhs in jobs:
                    h = 2 * pair + h01
                    nc.tensor.matmul(
                        av[0:MAV, pair, h01, :], vA[:, kt, h, :], rhs,
                        start=(kt == 0 and h01 == 0),
                        stop=(kt == 15), skip_group_check=True)

            def flush_av(keep=0):
                while len(pend_av) > keep:
                    emit_av_jobs(pend_av.pop(0))

            for m in range(8):
                for pair in range(2):
                    for h01 in range(2):
                        h = 2 * pair + h01
                        if m == g:
                            pend_av.append(
                                [(2 * m + s2, pair, h01,
                                  stg[:, h, 1 + s2, :, :])
                                 for s2 in range(2)])
                            flush_av(keep=DELAY)
                            continue
                        rs = slice(64 * h01, 64 * h01 + 64)
                        qx = qep.tile([128, 2, 256], F32, tag="qe", name="qx")
                        et = etp.tile([128, 2, 256], F16, tag="expT",
                                      name="et")
                        fss = []
                        for s2 in range(2):
                            kt = 2 * m + s2
                            var = "L" if kt < 2 * g else "R"
                            if kt == 2 * g - 1:
                                fs = slice(128, 256)   # far cols of group
                            elif kt == 2 * g + 2:
                                fs = slice(0, 128)
                            else:
                                fs = slice(0, 256)
                            fss.append(fs)
                            nc.tensor.matmul(
                                qx[:, s2, fs],
                                qkT[pair][var][rs, 128 * kt:128 * kt + 128],
                                qkT[pair]["q"][rs, g0 + fs.start:g0 + fs.stop],
                                start=True, stop=True)
                        flush_av(keep=DELAY)
                        if fss[0] == slice(0, 256) and fss[1] == slice(0, 256):
                            nc.scalar.activation(et[:], qx[:], AF.Exp)
                        else:
                            for s2 in range(2):
                                nc.scalar.activation(
                                    et[:, s2, fss[s2]], qx[:, s2, fss[s2]],
                                    AF.Exp)
                        for s2 in range(2):
                            kt = 2 * m + s2
                            if kt == 2 * g - 1:
                                nc.vector.tensor_copy(
                                    et[:, s2, 0:128], stg[:, h, 0, 0, :])
                            elif kt == 2 * g + 2:
                                nc.vector.tensor_copy(
                                    et[:, s2, 128:256], stg[:, h, 3, 1, :])
                        if DBG and g == DBG_G and m == 0:
                            nc.gpsimd.dma_start(
                                dbg_et[:, 512 * h:512 * h + 512],
                                et.rearrange("p a b -> p (a b)"))
                        pend_av.append([(2 * m + s2, pair, h01, et[:, s2, :])
                                        for s2 in range(2)])
            flush_av()
            return av

        def emit_finish_a(g, wctx, av):
            """Evacuate av to SBUF (frees the PSUM bank) and run the
            stats flips + sL/sR assembly (DVE + DMA only, no tensor)."""
            pkL, pkR = wctx["pkL"], wctx["pkR"]
            # all av rows to SBUF; rows 64.. also to DRAM for the flips
            stats = stp.tile([128, 2, 2, 256], F32, tag="stats", name="stats")
            nc.vector.tensor_copy(stats[0:113], av[0:113])
            sb0 = (g & 1) * 49 * 1024
            nc.sync.dma_start(
                AP(statd_t, sb0, [[1024, 49], [1, 1024]]), stats[64:113])
            Dcol = colp.tile([128, 8], F32, tag="Dcol", name="Dcol")
            TcolL = colp.tile([128, 8], F32, tag="TcolL", name="TcolL")
            TcolR = colp.tile([128, 8], F32, tag="TcolR", name="TcolR")
            subL8 = colp.tile([128, 8], F32, tag="subL8", name="subL8")
            subR8 = colp.tile([128, 8], F32, tag="subR8", name="subR8")
            for tq in range(2):
                t = 2 * g + tq
                # statd row r: 0=D, 1+kt=T_kt, 17+t=subL_t, 33+t=subR_t
                rL = t if t > 0 else 0        # T_{t-1}; junk for t=0
                rRr = t + 2 if t < 15 else 0  # T_{t+1}; junk for t=15
                for dst, r in ((Dcol, 0), (TcolL, rL), (TcolR, rRr),
                               (subL8, 17 + t), (subR8, 33 + t)):
                    nc.sync.dma_start(
                        dst[:, 4 * tq:4 * tq + 4].rearrange(
                            "p (a b) -> p a b", a=2),
                        AP(statd_t, sb0 + r * 1024 + 128 * tq,
                           [[1, 128], [512, 2], [256, 2]]))
            sL8 = colp.tile([128, 8], F32, tag="sL8", name="sL8")
            sR8 = colp.tile([128, 8], F32, tag="sR8", name="sR8")
            nc.vector.tensor_tensor(sL8[:], TcolL[:], steps_sb[:, 0, :],
                                    op=ALU.mult)
            nc.vector.tensor_add(sL8[:], sL8[:], subL8[:])
            nc.vector.tensor_add(sL8[:], sL8[:], pkL[:])
            nc.vector.tensor_tensor(sR8[:], TcolR[:], steps_sb[:, 1, :],
                                    op=ALU.mult)
            nc.vector.tensor_add(sR8[:], sR8[:], subR8[:])
            nc.vector.tensor_add(sR8[:], sR8[:], pkR[:])
            if g == 0:   # t=0 has no T_{t-1} term
                nc.vector.tensor_tensor(sL8[:, 0:4], subL8[:, 0:4],
                                        pkL[:, 0:4], op=ALU.add)
            if g == 7:   # t=15 has no T_{t+1} term
                nc.vector.tensor_tensor(sR8[:, 4:8], subR8[:, 4:8],
                                        pkR[:, 4:8], op=ALU.add)
            rec = colp.tile([128, 8], F32, tag="rec", name="rec")
            nc.vector.reciprocal(rec[:], Dcol[:])
            # pack to fp16 rows: chans [rec | sL | sR], each (pr, h01, tq)
            pkout = colp.tile([128, 24], F16, tag="pkout", name="pkout")
            for blk, src in ((0, rec), (8, sL8), (16, sR8)):
                nc.vector.tensor_copy(
                    pkout[:, blk:blk + 8].rearrange(
                        "p (a b c) -> p c a b", a=2, b=2),
                    src[:].rearrange("p (c a b) -> p c a b", c=2, a=2))
            rowout = colp.tile([1, 128, 24], F16, tag="rowout", name="rowout")
            nc.sync.dma_start(rowout.rearrange("o p c -> o (p c)"), pkout[:])
            if DBG and g == DBG_G:
                nc.sync.dma_start(dbg_av,
                                  stats.rearrange("p a b c -> p (a b c)"))
                nc.sync.dma_start(dbg_row, rowout.rearrange("o p c -> o (p c)"))
                nc.gpsimd.dma_start(
                    dbg_stg,
                    wctx["stg"].rearrange("p a b c d -> p (a b c d)"))
            return {"stats": stats, "rowout": rowout}

        def emit_finish_b(g, wctx, fctx):
            """rel-v matmuls, normalization, output projection."""
            arel16s = wctx["arel"]
            stats = fctx["stats"]
            rowout = fctx["rowout"]
            ct16 = {}
            for pair in range(2):
                # rel-v interior via transposed diagonal coefficients
                atp = wqp.tile([127, 2, 2, 128], F16, tag="wqps", name="atp")
                for h01 in range(2):
                    for tq in range(2):
                        nc.tensor.transpose(
                            atp[0:127, h01, tq, :],
                            arel16s[tq][:, 2 * pair + h01, 0:127], id_sb[:])
                arelTs = atsp.tile([127, 2, 2, 128], F16, tag="arelTs",
                                   name="arelTs")
                nc.vector.tensor_copy(arelTs[0:127], atp[0:127])
                if DBG and g == DBG_G:
                    nc.sync.dma_start(
                        dbg_ar[0:127, 512 * pair:512 * pair + 512],
                        arelTs.rearrange("p a b c -> p (a b c)"))
                relps = qep.tile([64, 2, 256], F32, tag="qe", name="relps")
                nc.tensor.matmul(
                    relps[:], rvm2_sb[0:127, :],
                    arelTs[0:127].rearrange("p a b c -> p (a b c)"),
                    start=True, stop=False)
                nc.tensor.matmul(
                    relps[:], rv0r_sb[:],
                    rowout[0:1, :, 8 + 4 * pair:12 + 4 * pair].rearrange(
                        "o p (a b) -> o a b p", a=2),
                    start=False, stop=False)
                nc.tensor.matmul(
                    relps[:], rvl_sb[:],
                    rowout[0:1, :, 16 + 4 * pair:20 + 4 * pair].rearrange(
                        "o p (a b) -> o a b p", a=2),
                    start=False, stop=True)
                bcps = qep.tile([64, 2, 256], F32, tag="qe", name="bcps")
                nc.tensor.matmul(
                    bcps[:], ones1[0:1, 0:64],
                    rowout[0:1, :, 4 * pair:4 * pair + 4].rearrange(
                        "o p (a b) -> o a b p", a=2),
                    start=True, stop=True)
                rbc = ctp.tile([64, 2, 256], F16, tag="rbc", name="rbc")
                nc.vector.tensor_copy(rbc[:], bcps[:])
                ctmp = ctp.tile([64, 2, 256], F16, tag="ctmp", name="ctmp")
                nc.vector.tensor_add(ctmp[:], stats[0:64, pair, :, :],
                                     relps[:])
                nc.vector.tensor_tensor(ctmp[:], ctmp[:], rbc[:],
                                        op=ALU.mult)
                nc.vector.tensor_add(ctmp[:], ctmp[:], bvbc_sb[:, pair, :, :])
                ct = ct16p.tile([128, 256], F16, tag=f"ct{pair}",
                                name=f"ct{pair}")
                nc.vector.tensor_copy(ct[0:64, :], ctmp[:, 0, :])
                nc.sync.dma_start(ct[64:128, :], ctmp[:, 1, :])
                ct16[pair] = ct

            for tq in range(2):
                out_sb = outp.tile([128, 1024], F16, tag="out_sb",
                                   name="out_sb")
                for nch in range(2):
                    op_ps = qep.tile([128, 512], F32, tag="qe", name="op_ps")
                    for pair in range(2):
                        nc.tensor.matmul(
                            op_ps[:], ct16[pair][:, 128 * tq:128 * tq + 128],
                            wout_sb[:, pair, 512 * nch:512 * nch + 512],
                            start=(pair == 0), stop=(pair == 1))
                    nc.vector.tensor_copy(
                        out_sb[:, 512 * nch:512 * nch + 512], op_ps[:])
                r0_ = 256 * g + 128 * tq
                nc.sync.dma_start(out[r0_:r0_ + 128, :], out_sb[:])

        # ---------------- phase 2: software-pipelined groups -------------
        # finish_a(g) (DVE/DMA stats evacuation) runs right after
        # ktloop(g); finish_b(g) (tensor) trails ktloop(g+1) so its
        # serial stats chain hides under the next group's tensor work
        wctx = emit_wstage(0)
        pend = None   # (g, wctx, fctx) awaiting finish_b
        for g in range(8):
            av = emit_ktloop(g, wctx)
            fctx = emit_finish_a(g, wctx, av)
            cur = (g, wctx, fctx)
            if g < 7:
                nxt = emit_wstage(g + 1)
            if pend is not None:
                emit_finish_b(*pend)
            pend = cur
            if g < 7:
                wctx = nxt
        emit_finish_b(*pend)

    nc.compile()
    return nc


def get_nc():
    if "nc" not in _cache:
        _cache["nc"] = _build()
    return _cache["nc"]


def shard_inputs(inputs):
    """Build per-core input maps from full inputs (layout prep only)."""
    x = np.asarray(inputs["x"], np.float32)
    W_qkv = np.asarray(inputs["W_qkv"], np.float32)
    b_qkv = np.asarray(inputs["b_qkv"], np.float32)
    W_out = np.asarray(inputs["W_out"], np.float32)
    rk = np.asarray(inputs["rel_emb_k"], np.float32)
    rv = np.asarray(inputs["rel_emb_v"], np.float32)

    Wq, Wk, Wv = W_qkv[:, 0:D], W_qkv[:, D:2 * D], W_qkv[:, 2 * D:3 * D]
    bqf, bkf, bvf = b_qkv[0:D], b_qkv[D:2 * D], b_qkv[2 * D:3 * D]

    cidx = np.clip(np.arange(512) - 256, -64, 64) + 64   # [512] vocab index
    relk_pad = rk.T[:, cidx].astype(np.float16)           # [64, 512]
    relk_host = np.concatenate([relk_pad, relk_pad], axis=0)  # [128, 512]
    rvm2_host = np.zeros((128, 64), np.float16)
    rvm2_host[0:127] = rv[1:128].astype(np.float16)
    rv0r_host = rv[0:1].astype(np.float16)
    rvl_host = rv[128:129].astype(np.float16)
    r0 = np.tile(rk[0], 2).reshape(128, 1)
    r1 = np.tile(rk[128], 2).reshape(128, 1)
    r01_host = np.concatenate([r0, r1], 1).astype(np.float32)

    # tail masks [128, 512] master: middle tiles slice [0:wlen],
    # t=0 slices [128:128+wlen]
    jj = np.arange(512)[None, :]
    ppi = np.arange(128)[:, None]
    ml_host = ((jj <= ppi + 64) &
               (jj >= 128 * (ppi >= 64))).astype(np.float16)
    mr_host = ((jj >= ppi + 192) &
               (jj <= 255 + 128 * (ppi >= 64))).astype(np.float16)
    ident_host = np.eye(128, dtype=np.float16)

    # augmented-V constant columns [kt, h, 49]:
    # col 0: ones (D); 1..16: eye16 (T_kt); 17..32: subL; 33..48: subR
    aug = np.zeros((16, 52), np.float16)
    aug[:, 0] = 1.0
    for kt in range(16):
        aug[kt, 1 + kt] = 1.0
        for t in range(16):
            if kt <= t - 2:
                aug[kt, 17 + t] = 1.0
            if kt >= t + 2:
                aug[kt, 33 + t] = 1.0
    import ml_dtypes
    F8H = ml_dtypes.float8_e4m3fn
    vaug_host = np.broadcast_to(
        np.repeat(aug[:, None, :], NH, axis=1).reshape(1, -1),
        (128, 16 * NH * 52)).astype(np.float16).copy()

    # step gates for the boundary T_kt term: L uses i_loc>=64, R uses <64
    iloc = np.arange(128)[:, None]
    st_hi = (iloc >= 64).astype(np.float32)
    st_lo = (iloc < 64).astype(np.float32)
    steps_host = np.concatenate(
        [np.repeat(st_hi, 8, 1), np.repeat(st_lo, 8, 1)], 1)

    in_maps = []
    for c in range(N_CORES):
        b, gg = c // 4, c % 4
        cols = slice(256 * gg, 256 * gg + 256)
        bvc = bvf[cols].reshape(2, 2, 64)          # [pair, h01, hd]
        bvbc_host = np.broadcast_to(
            bvc.transpose(2, 0, 1)[:, :, :, None],
            (64, 2, 2, 256)).reshape(64, 1024).astype(np.float16).copy()
        m = {
            "xT": np.ascontiguousarray(x[b].T),
            "wq": np.ascontiguousarray(Wq[:, cols]),
            "wk": np.ascontiguousarray(Wk[:, cols]),
            "wv": np.ascontiguousarray(Wv[:, cols]),
            "bq": np.ascontiguousarray(bqf[cols].reshape(2, 128).T),
            "bk": np.ascontiguousarray(bkf[cols].reshape(2, 128).T),
            "r01": r01_host,
            "relk": relk_host,
            "rvm2": rvm2_host,
            "rv0r": rv0r_host,
            "rvl": rvl_host,
            "wout": np.ascontiguousarray(
                W_out[cols].reshape(2, 128, 1024).transpose(1, 0, 2)
            ).astype(np.float16),
            "mlm": ml_host,
            "mrm": mr_host,
            "ident": ident_host,
            "vaug": vaug_host,
            "steps": steps_host,
            "bvbc": bvbc_host,
        }
        in_maps.append(m)
    return in_maps


def unshard_outputs(results, inputs):
    b_out = np.asarray(inputs["b_out"], np.float32)
    out = np.zeros((B, S, D), np.float32)
    for c in range(N_CORES):
        out[c // 4] += results[c]["out"].astype(np.float32)
    out += b_out[None, None, :]
    return out


def kernel(**inputs):
    from concourse import bass_utils
    nc = get_nc()
    in_maps = shard_inputs(inputs)
    res = bass_utils.run_bass_kernel_spmd(nc, in_maps, list(range(N_CORES)))
    return unshard_outputs(res.results, inputs)


if __name__ == "__main__":
    rng = np.random.default_rng(0)
    demo = {
        "x": rng.standard_normal((B, S, D)).astype(np.float32),
        "W_qkv": (rng.standard_normal((D, 3 * D)) * 0.02).astype(np.float32),
        "b_qkv": np.zeros(3 * D, np.float32),
        "W_out": (rng.standard_normal((D, D)) * 0.02).astype(np.float32),
        "b_out": np.zeros(D, np.float32),
        "rel_emb_k": (rng.standard_normal((VOC, HD)) * 0.02).astype(np.float32),
        "rel_emb_v": (rng.standard_normal((VOC, HD)) * 0.02).astype(np.float32),
    }
    o = kernel(**demo)
    print(o.shape, float(np.abs(o).max()))
